# revision 17
# baseline (speedup 1.0000x reference)
"""Self-contained TRN2 Bass kernel for nn_GATRotationRegressor.

kernel(**inputs) -> [16384, 24, 6] fp32. Data-parallel over 8 NeuronCores;
all layouts/shapes hardcoded for B=16384, J=24, H=128, heads=4, L=3.

v2: software-pipelined (W blocks interleaved), PSUM residual folding,
fused elu, LN gamma/beta folded into downstream matmuls, engine rebalance.
"""
from contextlib import ExitStack

import numpy as np

import concourse.bass as bass
import concourse.tile as tile
from concourse import mybir


PARENTS = [-1, 0, 0, 0, 1, 2, 3, 4, 5, 6, 7, 8, 9, 9, 9, 12, 13, 14, 16, 17, 18, 19, 20, 21]
B, J, IN_DIM, H, HEADS, OUT_DIM, L = 16384, 24, 3, 128, 4, 6, 3
C = H // HEADS
SLOPE = 0.2
KMAX = 5           # padded neighbor slots per dst
# feature permutation: device feature n = c*4 + h  <->  model feature o = h*32 + c
FPERM_O_OF_N = np.array([(n % HEADS) * C + n // HEADS for n in range(H)])
N_CORES = 8
BC = B // N_CORES  # graphs per core


def children(j):
    return [c for c, p in enumerate(PARENTS) if p == j]


def edge_slots():
    """For every real edge (src, dst) return its slot k at dst."""
    slots = {}
    for j in range(J):
        slots[(j, j)] = 0
        p = PARENTS[j]
        if p >= 0:
            slots[(p, j)] = 1
        for i, c in enumerate(children(j)):
            slots[(c, j)] = 2 + i
    return slots


def build_runs():
    """Greedy decomposition of the 70 edges into strided runs.

    Returns list of (src0, sstep, dst0, n, slot)."""
    slots = edge_slots()
    edges = sorted(slots.keys(), key=lambda e: (slots[e], e[1]))
    runs = []
    used = set()
    for e in edges:
        if e in used:
            continue
        src, dst = e
        k = slots[e]
        for sstep in (1, 0):
            n = 1
            while True:
                nxt = (src + sstep * n, dst + n)
                if nxt in slots and slots[nxt] == k and nxt not in used:
                    n += 1
                else:
                    break
            if n > 1 or sstep == 0:
                break
        for i in range(n):
            used.add((src + sstep * i, dst + i))
        runs.append((src, sstep, dst, n, k))
    assert sum(r[3] for r in runs) == 70, sum(r[3] for r in runs)
    return runs


def _edges():
    e = []
    for c, p in enumerate(PARENTS):
        if p >= 0:
            e.append((p, c)); e.append((c, p))
    for j in range(len(PARENTS)):
        e.append((j, j))
    a = np.asarray(e, dtype=np.int32)
    return a[:, 0], a[:, 1]


def _ln_np(x, g, b, eps=1e-5):
    m = x.mean(-1, keepdims=True)
    v = x.var(-1, keepdims=True)
    return (x - m) / np.sqrt(v + eps) * g + b


def np_reference(x, p, collect=None):
    """Numpy port of reference.py for an arbitrary batch."""
    Bn = x.shape[0]
    src, dst = _edges()
    h = x @ p["in_w"] + p["in_b"] + p["pos"][None]
    res = x @ p["res_w"] + p["res_b"]
    cc = collect if collect is not None else {}
    for l in range(L):
        hp = h
        xl = (h @ p["gat_w"][l]).reshape(Bn, J, HEADS, C)
        a_s = np.einsum('bjhc,hc->bjh', xl, p["att_s"][l])
        a_d = np.einsum('bjhc,hc->bjh', xl, p["att_d"][l])
        cc[f"xl{l}"] = xl; cc[f"a_s{l}"] = a_s; cc[f"a_d{l}"] = a_d
        e = a_s[:, src] + a_d[:, dst]
        e = np.where(e > 0, e, SLOPE * e)
        ex = np.exp(e)
        den = np.zeros((Bn, J, HEADS), e.dtype)
        np.add.at(den, (slice(None), dst), ex)
        alpha = ex / den[:, dst]
        cc[f"alpha{l}"] = alpha
        msg = xl[:, src] * alpha[..., None]
        out = np.zeros_like(xl)
        np.add.at(out, (slice(None), dst), msg)
        out = out.reshape(Bn, J, H) + p["gat_b"][l]
        cc[f"agg{l}"] = out
        out = np.where(out > 0, out, np.exp(np.minimum(out, 0)) - 1)  # elu
        out = _ln_np(out, p["ln_g"][l], p["ln_b"][l])
        cc[f"o{l}"] = out
        h = out + hp if l > 0 else out
    h = h + res
    y = np.maximum(h @ p["w1"] + p["b1"], 0)
    y = _ln_np(y, p["lng2"], p["lnb2"])
    return y @ p["w2"] + p["b2"]


def host_prep(inputs):
    """Host-side weight algebra: per-pass folded stationaries + biases."""
    f32 = np.float32
    in_w, in_b = np.asarray(inputs["in_w"]), np.asarray(inputs["in_b"])
    res_w, res_b = np.asarray(inputs["res_w"]), np.asarray(inputs["res_b"])
    pos = np.asarray(inputs["pos"])
    gat_w = np.asarray(inputs["gat_w"])
    att_s, att_d = np.asarray(inputs["att_s"]), np.asarray(inputs["att_d"])
    gat_b = np.asarray(inputs["gat_b"])
    ln_g, ln_b = np.asarray(inputs["ln_g"]), np.asarray(inputs["ln_b"])
    w1, b1 = np.asarray(inputs["w1"]), np.asarray(inputs["b1"])
    lng2, lnb2 = np.asarray(inputs["lng2"]), np.asarray(inputs["lnb2"])
    w2, b2 = np.asarray(inputs["w2"]), np.asarray(inputs["b2"])

    # per-layer score projections: ws[l] [H, 8] cols = (a_s h0..h3, a_d h0..h3)
    ws = np.zeros((L, H, 8), f32)
    for l in range(L):
        for h in range(HEADS):
            ws[l, :, h] = gat_w[l, :, h * C:(h + 1) * C] @ att_s[l, h]
            ws[l, :, 4 + h] = gat_w[l, :, h * C:(h + 1) * C] @ att_d[l, h]

    P = FPERM_O_OF_N

    # L0 folded stationaries on rhs28 = [x(3); ones(1); onehot_j(24)]
    W0p = np.zeros((28, H), f32)
    W0p[0:3] = in_w @ gat_w[0]
    W0p[3] = in_b @ gat_w[0]
    W0p[4:28] = pos @ gat_w[0]
    W0p = W0p[:, P]
    WS0p = np.zeros((28, 8), f32)
    WS0p[0:3] = in_w @ ws[0]
    WS0p[3] = in_b @ ws[0]
    WS0p[4:28] = pos @ ws[0]

    gat_w_d = gat_w[:, P][:, :, P]          # rows+cols permuted
    ws_d = ws[:, P]                          # rows permuted
    gat_b_d = gat_b[:, P]
    ln_g_d = ln_g[:, P]
    ln_b_d = ln_b[:, P]
    w1_d = w1[P, :]

    # Per-(layer, pass) stationaries with LN gamma folded in.
    # h_l = sum_{i<l} (g_i*o_i + lnb_i)   (o_i = LN-raw of layer i)
    # xl_l = gat_w_l^T h_l + (bias: gat_b_l + gat_w_l^T sum lnb_i)
    GS = np.zeros((L, L, H, H), f32)    # GS[l][i], valid i<l, l>=1
    WSP = np.zeros((L, L, H, 8), f32)
    XBIAS = np.zeros((L, H), f32)
    SBIAS = np.zeros((L, 8), f32)
    for l in range(L):
        lnb_sum = ln_b_d[:l].sum(axis=0) if l > 0 else np.zeros(H, f32)
        XBIAS[l] = gat_b_d[l] + lnb_sum @ gat_w_d[l]
        SBIAS[l] = lnb_sum @ ws_d[l]
        for i in range(l):
            GS[l, i] = ln_g_d[i][:, None] * gat_w_d[l]
            WSP[l, i] = ln_g_d[i][:, None] * ws_d[l]
    # score bias must be zero for E-build simplification (true: ln_b == 0)
    assert np.abs(SBIAS).max() == 0.0, "nonzero score bias not supported"

    # w1 consumed h_L + res; h_L = sum_i (g_i o_i + lnb_i)
    # doubled columns: psum rows 0:64 = z, rows 64:128 = z copy (squared at evac)
    W1S = np.zeros((L, H, H), f32)
    for i in range(L):
        w1s = ln_g_d[i][:, None] * w1_d
        W1S[i][:, 0:64] = w1s
        W1S[i][:, 64:128] = w1s
    lnb_sum = ln_b_d.sum(axis=0)
    RW1p = np.zeros((28, H), f32)
    rw = res_w @ w1
    RW1p[0:3, 0:64] = rw
    RW1p[0:3, 64:128] = rw
    rb1 = res_b @ w1 + lnb_sum @ w1_d        # b1 added at evac via ACT bias
    RW1p[3, 0:64] = rb1
    RW1p[3, 64:128] = rb1
    b1d = np.concatenate([b1, b1])
    # final LN2/w2 fold
    W2p = (lng2[:, None] * w2).astype(f32)          # [64, 6]
    c2 = W2p.sum(axis=0)                             # colsum for -mu*r term
    b2p = lnb2 @ w2 + b2                             # [6]

    return dict(
        W0p=W0p, WS0p=WS0p, GS=GS, WSP=WSP, XBIAS=XBIAS,
        W1S=W1S, RW1p=RW1p, W2p=W2p, c2=c2, b2p=b2p,
        ln_g=ln_g_d, ln_b=ln_b_d, b1=b1d,
    )


def make_rhs_const(G):
    """Rows 3..27 of rhs28: [ones; onehot_j] as [25, J*G] bf16."""
    import ml_dtypes
    N = J * G
    out = np.zeros((25, N), np.float32)
    out[0] = 1.0
    for j in range(J):
        out[1 + j, j * G:(j + 1) * G] = 1.0
    return out.astype(ml_dtypes.bfloat16)


def make_x_t(x_core, G):
    """x_core [BCk, 24, 3] -> x_t [3, BCk*24] bf16 with col = blk*G*24 + j*G + g."""
    import ml_dtypes
    BCk = x_core.shape[0]
    nblk = BCk // G
    xt = x_core.reshape(nblk, G, J, IN_DIM).transpose(3, 0, 2, 1).reshape(IN_DIM, BCk * J)
    return xt.astype(ml_dtypes.bfloat16)


F32 = mybir.dt.float32
BF16 = mybir.dt.bfloat16
AF = mybir.ActivationFunctionType
ALU = mybir.AluOpType
AX = mybir.AxisListType

RUNS = build_runs()
W = 2                 # software pipeline width (blocks in flight)


def rawap(t, off, dims):
    a = t[:]
    return bass.AP(tensor=a.tensor, offset=a.offset + off,
                   ap=[[a.ap[0][0], a.ap[0][1]]] + [list(d) for d in dims])


def emit_rsqrt(nc, out, in_, tmp, tmp2):
    """out = 1/sqrt(in_) via quake init + 2 Newton iters. All [128, F] F32."""
    I32 = mybir.dt.int32
    ib = in_.bitcast(I32)
    nc.vector.tensor_scalar(tmp.bitcast(I32), ib, 1, None,
                            op0=ALU.logical_shift_right)
    nc.vector.tensor_scalar(tmp.bitcast(I32), tmp.bitcast(I32), -1, 0x5F3759DF,
                            op0=ALU.mult, op1=ALU.add)
    for _ in range(2):
        nc.vector.tensor_tensor(out=tmp2, in0=tmp, in1=tmp, op=ALU.mult)
        nc.vector.tensor_tensor(out=tmp2, in0=tmp2, in1=in_, op=ALU.mult)
        nc.vector.tensor_scalar(tmp2, tmp2, -0.5, 1.5, op0=ALU.mult, op1=ALU.add)
        nc.vector.tensor_tensor(out=tmp, in0=tmp, in1=tmp2, op=ALU.mult)
    nc.vector.tensor_copy(out, tmp)


def kernel_body(ctx, tc, io, n_blocks, dbg_l=None):
    """io: dict name -> bass.AP (dram). Emits the kernel. G2=1 layout."""
    nc = tc.nc
    G = 128
    N = J * G            # 3072
    NB = n_blocks
    jD = 128             # per-joint column span
    aD = HEADS * KMAX    # 20: per-dst slot span in sE/sA
    eD = aD

    x_t, y_out = io["x_t"], io.get("y")

    pool = ctx.enter_context(tc.tile_pool(name="pool", bufs=1))
    consts = ctx.enter_context(tc.tile_pool(name="consts", bufs=1))
    psA = ctx.enter_context(tc.tile_pool(name="psA", bufs=2, space="PSUM"))
    psB = ctx.enter_context(tc.tile_pool(name="psB", bufs=3, space="PSUM"))

    def ps_small():
        return psB.tile([128, 512], F32, tag="ps_small", name="ps_small")

    # ---- persistent constants ----
    def cload(name, shape, dtype=BF16, src=None):
        t = consts.tile(list(shape), dtype, tag=f"c_{name}")
        nc.sync.dma_start(t[:], src if src is not None else io[name])
        return t

    c_W0p = cload("W0p", (28, H))
    c_WS0p = cload("WS0p", (28, 8))
    c_GS = {}
    c_WSP = {}
    for l in (1, 2):
        for i in range(l):
            c_GS[(l, i)] = cload(f"GS{l}{i}", (H, H), src=io["GS"][l][i])
            c_WSP[(l, i)] = cload(f"WSP{l}{i}", (H, 8), src=io["WSP"][l][i])
    c_W1S = [cload(f"W1S{i}", (H, H), src=io["W1S"][i]) for i in range(L)]
    c_RW1p = cload("RW1p", (28, H))
    c_W2p = cload("W2p", (H // 2, OUT_DIM))
    c_xb = [cload(f"xb{l}", (H, 1), F32, io["XBIAS"][l].unsqueeze(1))
            for l in range(L)]
    c_b1 = cload("b1", (H, 1), F32, io["B1"].unsqueeze(1))
    C2B2 = io["C2B2"]
    c_c2 = cload("c2", (128, OUT_DIM), F32,
                 bass.AP(tensor=C2B2.tensor, offset=C2B2.offset,
                         ap=[[0, 128], [1, OUT_DIM]]))
    c_b2p = cload("b2p", (128, OUT_DIM), F32,
                  bass.AP(tensor=C2B2.tensor, offset=C2B2.offset + OUT_DIM,
                          ap=[[0, 128], [1, OUT_DIM]]))
    c_ones = consts.tile([128, 1], BF16, tag="c_ones")
    nc.vector.memset(c_ones[:], 1.0)
    # packed [ones64;0 | 0;ones64] for z|z2 stats
    c_oz = consts.tile([128, 2], BF16, tag="c_oz")
    nc.vector.memset(c_oz[:], 0.0)
    nc.vector.memset(c_oz[0:64, 0:1], 1.0)
    nc.vector.memset(c_oz[64:128, 1:2], 1.0)

    def transpose(dst_t, src_t, eng):
        eng.dma_start_transpose(
            dst_t[:].rearrange("p (k q) -> p k q", q=128), src_t[:])

    def block_prog(blk):
        """Generator emitting one block's program; yields at stage breaks."""
        rhs28 = pool.tile([28, N], BF16, tag="rhs28", bufs=W)
        nc.sync.dma_start(rhs28[3:28, :], io["rhs_const"])
        nc.sync.dma_start(rhs28[0:3, :], x_t[:, blk * N:(blk + 1) * N])
        sE = pool.tile([128, J * eD], F32, tag="sE", bufs=W)
        nc.vector.memset(sE[:], -10000.0)
        o_fm = []
        yield

        for l in range(L):
            # ---- xl matmul (moving, multi-rhs PSUM accum) + ACT evac ----
            t_xlf = pool.tile([128, N], BF16, tag="t_xlf", bufs=W)
            for c0 in range(0, N, 1024):
                pt = psA.tile([128, 1024], F32, tag="pt_mm")
                npass = 1 if l == 0 else l
                for i in range(npass):
                    lhs = c_W0p if l == 0 else c_GS[(l, i)]
                    rhs_t = rhs28 if l == 0 else o_fm[i]
                    kdim = 28 if l == 0 else 128
                    for s0 in range(0, 1024, 512):
                        nc.tensor.matmul(pt[:, s0:s0 + 512], lhs[0:kdim, :],
                                         rhs_t[0:kdim, c0 + s0:c0 + s0 + 512],
                                         start=(i == 0), stop=(i == npass - 1))
                nc.scalar.activation(t_xlf[:, c0:c0 + 1024], pt[:],
                                     AF.Identity, bias=c_xb[l][:, 0:1],
                                     scale=1.0)
            # ---- scores (micro, stationary=data) -> psum [128, 192] ----
            ps_s = ps_small()
            for j in range(J):
                npass = 1 if l == 0 else l
                for i in range(npass):
                    lhs_t = rhs28 if l == 0 else o_fm[i]
                    kdim = 28 if l == 0 else 128
                    wmat = c_WS0p if l == 0 else c_WSP[(l, i)]
                    nc.tensor.matmul(ps_s[:, j * 8:(j + 1) * 8],
                                     lhs_t[0:kdim, j * jD:(j + 1) * jD],
                                     wmat[0:kdim, :],
                                     start=(i == 0), stop=(i == npass - 1))
            sS = pool.tile([128, J * 8], F32, tag="sS", bufs=W)
            nc.vector.tensor_copy(sS[:], ps_s[:, 0:J * 8])
            yield

            # ---- T1: xl FM -> GM ----
            t_xlg = pool.tile([128, N], BF16, tag="t_xlg", bufs=W)
            transpose(t_xlg, t_xlf, nc.sync)
            if dbg_l == l and "dbg_sS" in io:
                nc.sync.dma_start(io["dbg_sS"], sS[:])

            # ---- E build (gpsimd) ----
            for (src0, sstep, dst0, n, k) in RUNS:
                out_ap = rawap(sE, dst0 * eD + k * HEADS,
                               [(eD, n), (1, HEADS)])
                as_ap = rawap(sS, src0 * 8, [(8 * sstep, n), (1, HEADS)])
                ad_ap = rawap(sS, dst0 * 8 + 4, [(8, n), (1, HEADS)])
                nc.gpsimd.tensor_tensor(out=out_ap, in0=as_ap, in1=ad_ap,
                                        op=ALU.add)
            yield

            # ---- P = exp(lrelu(E)) (V 3-op + S exp); den; alpha ----
            sP = pool.tile([128, J * eD], F32, tag="sP", bufs=W)
            sP2 = pool.tile([128, J * eD], F32, tag="sP2", bufs=1)
            nc.vector.tensor_scalar(sP[:], sE[:], 0.0, SLOPE, op0=ALU.min,
                                    op1=ALU.mult)
            nc.vector.tensor_scalar_max(sP2[:], sE[:], 0.0)
            nc.vector.tensor_tensor(out=sP[:], in0=sP[:], in1=sP2[:],
                                    op=ALU.add)
            nc.scalar.activation(sP[:], sP[:], AF.Exp)
            sden = pool.tile([128, J * HEADS], F32, tag="sden", bufs=W)
            sdr = pool.tile([128, J * HEADS], F32, tag="sdr", bufs=W)
            nc.vector.tensor_reduce(
                out=sden[:].rearrange("p (d h) -> p d h", d=J),
                in_=rawap(sP, 0, [(eD, J), (1, HEADS), (HEADS, KMAX)]),
                axis=AX.X, op=ALU.add)
            nc.vector.reciprocal(sdr[:], sden[:])
            sA = pool.tile([128, J * eD], BF16, tag="sA", bufs=W)
            nc.gpsimd.tensor_tensor(
                out=rawap(sA, 0, [(eD, J), (HEADS, KMAX), (1, HEADS)]),
                in0=rawap(sP, 0, [(eD, J), (HEADS, KMAX), (1, HEADS)]),
                in1=rawap(sdr, 0, [(HEADS, J), (0, KMAX), (1, HEADS)]),
                op=ALU.mult)
            if dbg_l == l and "dbg_sA" in io:
                nc.sync.dma_start(io["dbg_sA"], sA[:])
            if dbg_l == l and "dbg_xlg" in io:
                nc.sync.dma_start(io["dbg_xlg"], t_xlg[:])
            yield

            # ---- aggregation (V) ----
            t_v = pool.tile([128, N], BF16, tag="t_v", bufs=W)

            def xl_ap(j0, sstep, n):
                return rawap(t_xlg, j0 * jD,
                             [(jD * sstep, n), (HEADS, C), (1, HEADS)])

            def al_ap(dst0, n, k):
                return rawap(sA, dst0 * aD + k * HEADS,
                             [(aD, n), (0, C), (1, HEADS)])

            def v_ap(dst0, n, buf):
                return rawap(buf, dst0 * jD, [(jD, n), (HEADS, C), (1, HEADS)])

            for ri, (src0, sstep, dst0, n, k) in enumerate(RUNS):
                if ri == 0:
                    nc.vector.tensor_tensor(out=v_ap(0, 24, t_v),
                                            in0=xl_ap(0, 1, 24),
                                            in1=al_ap(0, 24, 0), op=ALU.mult)
                    continue
                t_tmp = pool.tile([128, N], BF16, tag="t_tmp", bufs=1)
                nc.vector.tensor_tensor(out=v_ap(dst0, n, t_tmp),
                                        in0=xl_ap(src0, sstep, n),
                                        in1=al_ap(dst0, n, k), op=ALU.mult)
                nc.vector.tensor_tensor(out=v_ap(dst0, n, t_v),
                                        in0=v_ap(dst0, n, t_v),
                                        in1=v_ap(dst0, n, t_tmp), op=ALU.add)
            if dbg_l == l and "dbg_v" in io:
                nc.sync.dma_start(io["dbg_v"], t_v[:])
            yield

            # ---- elu: w = max(v, min(exp(v),1)-1) ----
            e32 = pool.tile([128, N], F32, tag="e32", bufs=1)
            nc.scalar.activation(e32[:], t_v[:], AF.Exp)
            t_e1 = pool.tile([128, N], BF16, tag="t_e1", bufs=1)
            nc.gpsimd.tensor_scalar(t_e1[:], e32[:], 1.0, -1.0, op0=ALU.min,
                                    op1=ALU.add)
            t_w = pool.tile([128, N], BF16, tag="t_w", bufs=W)
            nc.vector.tensor_tensor(out=t_w[:], in0=t_v[:], in1=t_e1[:],
                                    op=ALU.max)
            if dbg_l == l and "dbg_w" in io:
                nc.sync.dma_start(io["dbg_w"], t_w[:])
            yield

            # ---- T2 + square (gpsimd) + stats micro-matmuls ----
            t_wf = pool.tile([128, N], BF16, tag="t_wf", bufs=W)
            transpose(t_wf, t_w, nc.scalar)
            t_w2f = pool.tile([128, N], BF16, tag="t_w2f", bufs=1)
            nc.gpsimd.tensor_tensor(out=t_w2f[:], in0=t_wf[:], in1=t_wf[:],
                                    op=ALU.mult)
            ps_t = ps_small()
            for j in range(J):
                nc.tensor.matmul(ps_t[:, j * 2:j * 2 + 1],
                                 t_wf[:, j * jD:(j + 1) * jD],
                                 c_ones[:], start=True, stop=True)
                nc.tensor.matmul(ps_t[:, j * 2 + 1:j * 2 + 2],
                                 t_w2f[:, j * jD:(j + 1) * jD],
                                 c_ones[:], start=True, stop=True)
            sst = pool.tile([128, J * 2], F32, tag="sst", bufs=W)
            nc.vector.tensor_copy(sst[:], ps_t[:, 0:J * 2])
            yield

            # ---- LN smalls: mu, rstd; bf16 x4-replicated ----
            smu = pool.tile([128, J], F32, tag="smu", bufs=1)
            svar = pool.tile([128, J], F32, tag="svar", bufs=1)
            sr = pool.tile([128, J], F32, tag="sr", bufs=1)
            sm2 = pool.tile([128, J], F32, tag="sm2", bufs=1)
            st1 = pool.tile([128, J], F32, tag="st1", bufs=1)
            st2 = pool.tile([128, J], F32, tag="st2", bufs=1)
            stv = sst[:].rearrange("p (m s) -> p m s", s=2)
            nc.vector.tensor_scalar_mul(smu[:], stv[:, :, 0], 1.0 / H)
            nc.vector.tensor_scalar_mul(svar[:], stv[:, :, 1], 1.0 / H)
            nc.vector.tensor_tensor(out=sm2[:], in0=smu[:], in1=smu[:],
                                    op=ALU.mult)
            nc.vector.tensor_tensor(out=svar[:], in0=svar[:], in1=sm2[:],
                                    op=ALU.subtract)
            nc.vector.tensor_scalar_add(svar[:], svar[:], 1e-5)
            emit_rsqrt(nc, sr[:], svar[:], st1[:], st2[:])
            nc.vector.tensor_tensor(out=sm2[:], in0=smu[:], in1=sr[:],
                                    op=ALU.mult)
            # replicate x4 in bf16 for 2x-mode broadcast TTs
            sr4 = pool.tile([128, J * 4], BF16, tag="sr4", bufs=W)
            sm24 = pool.tile([128, J * 4], BF16, tag="sm24", bufs=W)
            nc.vector.tensor_copy(rawap(sr4, 0, [(4, J), (1, 4)]),
                                  rawap(sr, 0, [(1, J), (0, 4)]))
            nc.vector.tensor_copy(rawap(sm24, 0, [(4, J), (1, 4)]),
                                  rawap(sm2, 0, [(1, J), (0, 4)]))

            # ---- LNraw = w*r - mu*r (V, 2x via x4-replicated bf16) ----
            t_lnr = pool.tile([128, N], BF16, tag="t_lnr", bufs=W)
            rb = rawap(sr4, 0, [(4, J), (0, C), (1, HEADS)])
            m2b = rawap(sm24, 0, [(4, J), (0, C), (1, HEADS)])
            v3 = rawap(t_lnr, 0, [(jD, J), (HEADS, C), (1, HEADS)])
            w3 = rawap(t_w, 0, [(jD, J), (HEADS, C), (1, HEADS)])
            nc.vector.tensor_tensor(out=v3, in0=w3, in1=rb, op=ALU.mult)
            nc.vector.tensor_tensor(out=v3, in0=v3, in1=m2b, op=ALU.subtract)
            yield

            # ---- T3: o_l = LNraw in FM ----
            o_l = pool.tile([128, N], BF16, tag=f"o{l}", bufs=W)
            transpose(o_l, t_lnr, nc.scalar)
            o_fm.append(o_l)
            if dbg_l == l and "dbg_h" in io:
                nc.sync.dma_start(io["dbg_h"], o_l[:])
            yield

        # ================= final MLP =================
        t_zz = pool.tile([128, N], BF16, tag="t_zz", bufs=W)
        for c0 in range(0, N, 1024):
            pt = psA.tile([128, 1024], F32, tag="pt_mm")
            for s0 in range(0, 1024, 512):
                for i in range(L):
                    nc.tensor.matmul(pt[:, s0:s0 + 512], c_W1S[i][:],
                                     o_fm[i][:, c0 + s0:c0 + s0 + 512],
                                     start=(i == 0), stop=False)
                nc.tensor.matmul(pt[:, s0:s0 + 512], c_RW1p[:],
                                 rhs28[:, c0 + s0:c0 + s0 + 512],
                                 start=False, stop=True)
            nc.scalar.activation(t_zz[:, c0:c0 + 1024], pt[:],
                                 AF.Relu, bias=c_b1[:, 0:1], scale=1.0)
        nc.gpsimd.tensor_tensor(out=t_zz[64:128, :], in0=t_zz[64:128, :],
                                in1=t_zz[64:128, :], op=ALU.mult)
        yield

        # packed z|z2 stats: one LDW + one FD=2 matmul per joint
        ps_t = ps_small()
        for j in range(J):
            nc.tensor.matmul(ps_t[:, j * 2:j * 2 + 2],
                             t_zz[:, j * jD:(j + 1) * jD],
                             c_oz[:], start=True, stop=True)
        sstz = pool.tile([128, J * 2], F32, tag="sstz", bufs=W)
        nc.vector.tensor_copy(sstz[:], ps_t[:, 0:J * 2])

        smu = pool.tile([128, J], F32, tag="zmu", bufs=1)
        svar = pool.tile([128, J], F32, tag="zvar", bufs=1)
        sr = pool.tile([128, J], F32, tag="zsr", bufs=W)
        sm2 = pool.tile([128, J], F32, tag="zsm2", bufs=W)
        st1 = pool.tile([128, J], F32, tag="zst1", bufs=1)
        st2 = pool.tile([128, J], F32, tag="zst2", bufs=1)
        stv = sstz[:].rearrange("p (m s) -> p m s", s=2)
        nc.vector.tensor_scalar_mul(smu[:], stv[:, :, 0], 2.0 / H)
        nc.vector.tensor_scalar_mul(svar[:], stv[:, :, 1], 2.0 / H)
        nc.vector.tensor_tensor(out=sm2[:], in0=smu[:], in1=smu[:],
                                op=ALU.mult)
        nc.vector.tensor_tensor(out=svar[:], in0=svar[:], in1=sm2[:],
                                op=ALU.subtract)
        nc.vector.tensor_scalar_add(svar[:], svar[:], 1e-5)
        emit_rsqrt(nc, sr[:], svar[:], st1[:], st2[:])
        nc.vector.tensor_tensor(out=sm2[:], in0=smu[:], in1=sr[:],
                                op=ALU.mult)
        yield

        # st6: y-partial = z^T @ W2p per joint -> [128 cols, 6]
        ps_y = ps_small()
        for j in range(J):
            nc.tensor.matmul(ps_y[:, j * OUT_DIM:(j + 1) * OUT_DIM],
                             t_zz[0:64, j * jD:(j + 1) * jD],
                             c_W2p[:], start=True, stop=True)
        st6 = pool.tile([128, J * OUT_DIM], F32, tag="st6", bufs=1)
        nc.vector.tensor_copy(st6[:], ps_y[:, 0:J * OUT_DIM])

        sy = pool.tile([128, J * OUT_DIM], F32, tag="sy", bufs=W)
        t6v = st6[:].rearrange("p (j o) -> p j o", j=J)
        yv = sy[:].rearrange("p (j o) -> p j o", j=J)
        rb = sr[:].unsqueeze(2).broadcast_to((128, J, OUT_DIM))
        m2b = sm2[:].unsqueeze(2).broadcast_to((128, J, OUT_DIM))
        c2b = rawap(c_c2, 0, [(0, J), (1, OUT_DIM)])
        b2b = rawap(c_b2p, 0, [(0, J), (1, OUT_DIM)])
        nc.vector.tensor_tensor(out=yv, in0=t6v, in1=rb, op=ALU.mult)
        nc.vector.tensor_tensor(out=t6v, in0=m2b, in1=c2b, op=ALU.mult)
        nc.vector.tensor_tensor(out=yv, in0=yv, in1=t6v, op=ALU.subtract)
        nc.vector.tensor_tensor(out=yv, in0=yv, in1=b2b, op=ALU.add)

        if y_out is not None:
            yo = bass.AP(
                tensor=y_out.tensor,
                offset=y_out.offset + blk * G * J * OUT_DIM,
                ap=[[J * OUT_DIM, 128], [OUT_DIM, J], [1, OUT_DIM]])
            nc.sync.dma_start(yo, sy[:].rearrange("p (j o) -> p j o", j=J))
        yield

    # ---- software-pipelined emission: W blocks interleaved ----
    for bb in range(0, NB, W):
        gens = [block_prog(bb + w) for w in range(min(W, NB - bb))]
        alive = list(gens)
        while alive:
            nxt = []
            for g in alive:
                try:
                    next(g)
                    nxt.append(g)
                except StopIteration:
                    pass
            alive = nxt


# ======================================================================
# Host driver: kernel(**inputs) -> np.ndarray
# ======================================================================
G2_FULL = 1
NB_FULL = (B // N_CORES) // 128


def _install_ntff_shim():
    import sys, types
    if "antenv.axon_hooks" in sys.modules:
        return
    mod = types.ModuleType("antenv.axon_hooks")
    mod._hook = None
    mod.set_axon_ntff_profile_hook = lambda h: setattr(mod, "_hook", h)
    mod.get_axon_ntff_profile_hook = lambda: mod._hook
    sys.modules["antenv.axon_hooks"] = mod
    try:
        from trn_agent_boot.trn_boot import _ntff_profile_via_ctypes
        mod.set_axon_ntff_profile_hook(
            _ntff_profile_via_ctypes("/opt/axon/libaxon_pjrt.so"))
    except Exception:
        pass
    try:
        import concourse.bass_utils as bu
        bu.upload_artifacts = lambda tmpdir: tmpdir
    except Exception:
        pass


_NC_CACHE = {}


def _build_nc(NB):
    key = NB
    if key in _NC_CACHE:
        return _NC_CACHE[key]
    from contextlib import ExitStack
    from concourse import bacc
    nc = bacc.Bacc("TRN2", target_bir_lowering=False, debug=False,
                   num_devices=N_CORES)
    G = 128
    BCk = G * NB
    dt = nc.dram_tensor
    io = {}
    def din(name, shape, dtype):
        io[name] = dt(name, shape, dtype, kind="ExternalInput").ap()
    din("x_t", (IN_DIM, BCk * J), BF16)
    din("rhs_const", (25, J * G), BF16)
    din("W0p", (28, H), BF16)
    din("WS0p", (28, 8), BF16)
    din("GS", (L, L, H, H), BF16)
    din("WSP", (L, L, H, 8), BF16)
    din("XBIAS", (L, H), F32)
    din("W1S", (L, H, H), BF16)
    din("RW1p", (28, H), BF16)
    din("B1", (H,), F32)
    din("W2p", (H // 2, OUT_DIM), BF16)
    din("C2B2", (2, OUT_DIM), F32)
    io["y"] = dt("y", (BCk, J, OUT_DIM), F32, kind="ExternalOutput").ap()
    with tile.TileContext(nc) as tc:
        with ExitStack() as ctx:
            kernel_body(ctx, tc, io, NB)
    nc.compile()
    _NC_CACHE[key] = nc
    return nc


def make_in_maps(inputs, NB=NB_FULL):
    import ml_dtypes
    bf = ml_dtypes.bfloat16
    pp = host_prep(inputs)
    G = 128
    BCk = G * NB
    x = np.asarray(inputs["x"], dtype=np.float32)
    shared = dict(
        rhs_const=np.ascontiguousarray(make_rhs_const(G)),
        W0p=np.ascontiguousarray(pp["W0p"].astype(bf)),
        WS0p=np.ascontiguousarray(pp["WS0p"].astype(bf)),
        GS=np.ascontiguousarray(pp["GS"].astype(bf)),
        WSP=np.ascontiguousarray(pp["WSP"].astype(bf)),
        XBIAS=np.ascontiguousarray(pp["XBIAS"].astype(np.float32)),
        W1S=np.ascontiguousarray(pp["W1S"].astype(bf)),
        RW1p=np.ascontiguousarray(pp["RW1p"].astype(bf)),
        B1=np.ascontiguousarray(pp["b1"].astype(np.float32)),
        W2p=np.ascontiguousarray(pp["W2p"].astype(bf)),
        C2B2=np.ascontiguousarray(
            np.stack([pp["c2"], pp["b2p"]]).astype(np.float32)),
    )
    in_maps = []
    for core in range(N_CORES):
        xc = x[core * BCk:(core + 1) * BCk]
        m = dict(shared)
        m["x_t"] = np.ascontiguousarray(make_x_t(xc, G))
        in_maps.append(m)
    return in_maps


def run_on_cores(inputs, NB=NB_FULL, trace=False):
    _install_ntff_shim()
    from concourse.bass_utils import run_bass_kernel_spmd
    nc = _build_nc(NB)
    in_maps = make_in_maps(inputs, NB)
    res = run_bass_kernel_spmd(nc, in_maps, list(range(N_CORES)), trace=trace)
    ys = [res.results[c]["y"] for c in range(N_CORES)]
    y = np.concatenate(ys, axis=0).astype(np.float32)
    return y, res


def kernel(**inputs) -> np.ndarray:
    y, _ = run_on_cores(inputs)
    return y


# revision 18
# speedup vs baseline: 1.9064x; 1.9064x over previous
"""Self-contained TRN2 Bass kernel for nn_GATRotationRegressor.

kernel(**inputs) -> [16384, 24, 6] fp32. Data-parallel over 8 NeuronCores;
all layouts/shapes hardcoded for B=16384, J=24, H=128, heads=4, L=3.

v2: software-pipelined (W blocks interleaved), PSUM residual folding,
fused elu, LN gamma/beta folded into downstream matmuls, engine rebalance.
"""
from contextlib import ExitStack

import numpy as np

import concourse.bass as bass
import concourse.tile as tile
from concourse import mybir


PARENTS = [-1, 0, 0, 0, 1, 2, 3, 4, 5, 6, 7, 8, 9, 9, 9, 12, 13, 14, 16, 17, 18, 19, 20, 21]
B, J, IN_DIM, H, HEADS, OUT_DIM, L = 16384, 24, 3, 128, 4, 6, 3
C = H // HEADS
SLOPE = 0.2
KMAX = 5           # padded neighbor slots per dst
# feature permutation: device feature n = c*4 + h  <->  model feature o = h*32 + c
FPERM_O_OF_N = np.array([(n % HEADS) * C + n // HEADS for n in range(H)])
N_CORES = 8
BC = B // N_CORES  # graphs per core


def children(j):
    return [c for c, p in enumerate(PARENTS) if p == j]


def edge_slots():
    """For every real edge (src, dst) return its slot k at dst."""
    slots = {}
    for j in range(J):
        slots[(j, j)] = 0
        p = PARENTS[j]
        if p >= 0:
            slots[(p, j)] = 1
        for i, c in enumerate(children(j)):
            slots[(c, j)] = 2 + i
    return slots


def build_runs():
    """Greedy decomposition of the 70 edges into strided runs.

    Returns list of (src0, sstep, dst0, n, slot)."""
    slots = edge_slots()
    edges = sorted(slots.keys(), key=lambda e: (slots[e], e[1]))
    runs = []
    used = set()
    for e in edges:
        if e in used:
            continue
        src, dst = e
        k = slots[e]
        for sstep in (1, 0):
            n = 1
            while True:
                nxt = (src + sstep * n, dst + n)
                if nxt in slots and slots[nxt] == k and nxt not in used:
                    n += 1
                else:
                    break
            if n > 1 or sstep == 0:
                break
        for i in range(n):
            used.add((src + sstep * i, dst + i))
        runs.append((src, sstep, dst, n, k))
    assert sum(r[3] for r in runs) == 70, sum(r[3] for r in runs)
    return runs


def _edges():
    e = []
    for c, p in enumerate(PARENTS):
        if p >= 0:
            e.append((p, c)); e.append((c, p))
    for j in range(len(PARENTS)):
        e.append((j, j))
    a = np.asarray(e, dtype=np.int32)
    return a[:, 0], a[:, 1]


def _ln_np(x, g, b, eps=1e-5):
    m = x.mean(-1, keepdims=True)
    v = x.var(-1, keepdims=True)
    return (x - m) / np.sqrt(v + eps) * g + b


def np_reference(x, p, collect=None):
    """Numpy port of reference.py for an arbitrary batch."""
    Bn = x.shape[0]
    src, dst = _edges()
    h = x @ p["in_w"] + p["in_b"] + p["pos"][None]
    res = x @ p["res_w"] + p["res_b"]
    cc = collect if collect is not None else {}
    for l in range(L):
        hp = h
        xl = (h @ p["gat_w"][l]).reshape(Bn, J, HEADS, C)
        a_s = np.einsum('bjhc,hc->bjh', xl, p["att_s"][l])
        a_d = np.einsum('bjhc,hc->bjh', xl, p["att_d"][l])
        cc[f"xl{l}"] = xl; cc[f"a_s{l}"] = a_s; cc[f"a_d{l}"] = a_d
        e = a_s[:, src] + a_d[:, dst]
        e = np.where(e > 0, e, SLOPE * e)
        ex = np.exp(e)
        den = np.zeros((Bn, J, HEADS), e.dtype)
        np.add.at(den, (slice(None), dst), ex)
        alpha = ex / den[:, dst]
        cc[f"alpha{l}"] = alpha
        msg = xl[:, src] * alpha[..., None]
        out = np.zeros_like(xl)
        np.add.at(out, (slice(None), dst), msg)
        out = out.reshape(Bn, J, H) + p["gat_b"][l]
        cc[f"agg{l}"] = out
        out = np.where(out > 0, out, np.exp(np.minimum(out, 0)) - 1)  # elu
        out = _ln_np(out, p["ln_g"][l], p["ln_b"][l])
        cc[f"o{l}"] = out
        h = out + hp if l > 0 else out
    h = h + res
    y = np.maximum(h @ p["w1"] + p["b1"], 0)
    y = _ln_np(y, p["lng2"], p["lnb2"])
    return y @ p["w2"] + p["b2"]


def host_prep(inputs):
    """Host-side weight algebra: per-pass folded stationaries + biases."""
    f32 = np.float32
    in_w, in_b = np.asarray(inputs["in_w"]), np.asarray(inputs["in_b"])
    res_w, res_b = np.asarray(inputs["res_w"]), np.asarray(inputs["res_b"])
    pos = np.asarray(inputs["pos"])
    gat_w = np.asarray(inputs["gat_w"])
    att_s, att_d = np.asarray(inputs["att_s"]), np.asarray(inputs["att_d"])
    gat_b = np.asarray(inputs["gat_b"])
    ln_g, ln_b = np.asarray(inputs["ln_g"]), np.asarray(inputs["ln_b"])
    w1, b1 = np.asarray(inputs["w1"]), np.asarray(inputs["b1"])
    lng2, lnb2 = np.asarray(inputs["lng2"]), np.asarray(inputs["lnb2"])
    w2, b2 = np.asarray(inputs["w2"]), np.asarray(inputs["b2"])

    # per-layer score projections: ws[l] [H, 8] cols = (a_s h0..h3, a_d h0..h3)
    ws = np.zeros((L, H, 8), f32)
    for l in range(L):
        for h in range(HEADS):
            ws[l, :, h] = gat_w[l, :, h * C:(h + 1) * C] @ att_s[l, h]
            ws[l, :, 4 + h] = gat_w[l, :, h * C:(h + 1) * C] @ att_d[l, h]

    P = FPERM_O_OF_N

    # L0 folded stationaries on rhs28 = [x(3); ones(1); onehot_j(24)]
    W0p = np.zeros((28, H), f32)
    W0p[0:3] = in_w @ gat_w[0]
    W0p[3] = in_b @ gat_w[0]
    W0p[4:28] = pos @ gat_w[0]
    W0p = W0p[:, P]
    WS0p = np.zeros((28, 8), f32)
    WS0p[0:3] = in_w @ ws[0]
    WS0p[3] = in_b @ ws[0]
    WS0p[4:28] = pos @ ws[0]

    gat_w_d = gat_w[:, P][:, :, P]          # rows+cols permuted
    ws_d = ws[:, P]                          # rows permuted
    gat_b_d = gat_b[:, P]
    ln_g_d = ln_g[:, P]
    ln_b_d = ln_b[:, P]
    w1_d = w1[P, :]

    # Per-(layer, pass) stationaries with LN gamma folded in.
    # h_l = sum_{i<l} (g_i*o_i + lnb_i)   (o_i = LN-raw of layer i)
    # xl_l = gat_w_l^T h_l + (bias: gat_b_l + gat_w_l^T sum lnb_i)
    GS = np.zeros((L, L, H, H), f32)    # GS[l][i], valid i<l, l>=1
    WSP = np.zeros((L, L, H, 8), f32)
    XBIAS = np.zeros((L, H), f32)
    SBIAS = np.zeros((L, 8), f32)
    for l in range(L):
        lnb_sum = ln_b_d[:l].sum(axis=0) if l > 0 else np.zeros(H, f32)
        XBIAS[l] = gat_b_d[l] + lnb_sum @ gat_w_d[l]
        SBIAS[l] = lnb_sum @ ws_d[l]
        for i in range(l):
            GS[l, i] = ln_g_d[i][:, None] * gat_w_d[l]
            WSP[l, i] = ln_g_d[i][:, None] * ws_d[l]
    # score bias must be zero for E-build simplification (true: ln_b == 0)
    assert np.abs(SBIAS).max() == 0.0, "nonzero score bias not supported"

    # w1 consumed h_L + res; h_L = sum_i (g_i o_i + lnb_i)
    # doubled columns: psum rows 0:64 = z, rows 64:128 = z copy (squared at evac)
    W1S = np.zeros((L, H, H), f32)
    for i in range(L):
        w1s = ln_g_d[i][:, None] * w1_d
        W1S[i][:, 0:64] = w1s
        W1S[i][:, 64:128] = w1s
    lnb_sum = ln_b_d.sum(axis=0)
    RW1p = np.zeros((28, H), f32)
    rw = res_w @ w1
    RW1p[0:3, 0:64] = rw
    RW1p[0:3, 64:128] = rw
    rb1 = res_b @ w1 + lnb_sum @ w1_d        # b1 added at evac via ACT bias
    RW1p[3, 0:64] = rb1
    RW1p[3, 64:128] = rb1
    b1d = np.concatenate([b1, b1])
    # final LN2/w2 fold
    W2p = (lng2[:, None] * w2).astype(f32)          # [64, 6]
    c2 = W2p.sum(axis=0)                             # colsum for -mu*r term
    b2p = lnb2 @ w2 + b2                             # [6]

    return dict(
        W0p=W0p, WS0p=WS0p, GS=GS, WSP=WSP, XBIAS=XBIAS,
        W1S=W1S, RW1p=RW1p, W2p=W2p, c2=c2, b2p=b2p,
        ln_g=ln_g_d, ln_b=ln_b_d, b1=b1d,
    )


def make_rhs_const(G):
    """Rows 3..27 of rhs28: [ones; onehot_j] as [25, J*G] bf16."""
    import ml_dtypes
    N = J * G
    out = np.zeros((25, N), np.float32)
    out[0] = 1.0
    for j in range(J):
        out[1 + j, j * G:(j + 1) * G] = 1.0
    return out.astype(ml_dtypes.bfloat16)


def make_x_t(x_core, G):
    """x_core [BCk, 24, 3] -> x_t [3, BCk*24] bf16 with col = blk*G*24 + j*G + g."""
    import ml_dtypes
    BCk = x_core.shape[0]
    nblk = BCk // G
    xt = x_core.reshape(nblk, G, J, IN_DIM).transpose(3, 0, 2, 1).reshape(IN_DIM, BCk * J)
    return xt.astype(ml_dtypes.bfloat16)


F32 = mybir.dt.float32
BF16 = mybir.dt.bfloat16
AF = mybir.ActivationFunctionType
ALU = mybir.AluOpType
AX = mybir.AxisListType

RUNS = build_runs()
W = 2                 # software pipeline width (blocks in flight)


def rawap(t, off, dims):
    a = t[:]
    return bass.AP(tensor=a.tensor, offset=a.offset + off,
                   ap=[[a.ap[0][0], a.ap[0][1]]] + [list(d) for d in dims])


def emit_rsqrt(nc, out, in_, tmp, tmp2):
    """out = 1/sqrt(in_) via quake init + 2 Newton iters. All [128, F] F32."""
    I32 = mybir.dt.int32
    ib = in_.bitcast(I32)
    nc.vector.tensor_scalar(tmp.bitcast(I32), ib, 1, None,
                            op0=ALU.logical_shift_right)
    nc.vector.tensor_scalar(tmp.bitcast(I32), tmp.bitcast(I32), -1, 0x5F3759DF,
                            op0=ALU.mult, op1=ALU.add)
    for _ in range(2):
        nc.vector.tensor_tensor(out=tmp2, in0=tmp, in1=tmp, op=ALU.mult)
        nc.vector.tensor_tensor(out=tmp2, in0=tmp2, in1=in_, op=ALU.mult)
        nc.vector.tensor_scalar(tmp2, tmp2, -0.5, 1.5, op0=ALU.mult, op1=ALU.add)
        nc.vector.tensor_tensor(out=tmp, in0=tmp, in1=tmp2, op=ALU.mult)
    nc.vector.tensor_copy(out, tmp)


def kernel_body(ctx, tc, io, n_blocks, dbg_l=None):
    """io: dict name -> bass.AP (dram). Emits the kernel. G2=1 layout."""
    nc = tc.nc
    G = 128
    N = J * G            # 3072
    NB = n_blocks
    jD = 128             # per-joint column span
    aD = HEADS * KMAX    # 20: per-dst slot span in sE/sA
    eD = aD

    x_t, y_out = io["x_t"], io.get("y")

    pool = ctx.enter_context(tc.tile_pool(name="pool", bufs=1))
    consts = ctx.enter_context(tc.tile_pool(name="consts", bufs=1))
    psA = ctx.enter_context(tc.tile_pool(name="psA", bufs=2, space="PSUM"))
    psB = ctx.enter_context(tc.tile_pool(name="psB", bufs=3, space="PSUM"))

    def ps_small():
        return psB.tile([128, 512], F32, tag="ps_small", name="ps_small")

    # ---- persistent constants ----
    def cload(name, shape, dtype=BF16, src=None):
        t = consts.tile(list(shape), dtype, tag=f"c_{name}")
        nc.sync.dma_start(t[:], src if src is not None else io[name])
        return t

    c_W0p = cload("W0p", (28, H))
    c_WS0p = cload("WS0p", (28, 8))
    c_GS = {}
    c_WSP = {}
    for l in (1, 2):
        for i in range(l):
            c_GS[(l, i)] = cload(f"GS{l}{i}", (H, H), src=io["GS"][l][i])
            c_WSP[(l, i)] = cload(f"WSP{l}{i}", (H, 8), src=io["WSP"][l][i])
    c_W1S = [cload(f"W1S{i}", (H, H), src=io["W1S"][i]) for i in range(L)]
    c_RW1p = cload("RW1p", (28, H))
    c_W2p = cload("W2p", (H // 2, OUT_DIM))
    c_xb = [cload(f"xb{l}", (H, 1), F32, io["XBIAS"][l].unsqueeze(1))
            for l in range(L)]
    c_b1 = cload("b1", (H, 1), F32, io["B1"].unsqueeze(1))
    C2B2 = io["C2B2"]
    c_c2 = cload("c2", (128, OUT_DIM), F32,
                 bass.AP(tensor=C2B2.tensor, offset=C2B2.offset,
                         ap=[[0, 128], [1, OUT_DIM]]))
    c_b2p = cload("b2p", (128, OUT_DIM), F32,
                  bass.AP(tensor=C2B2.tensor, offset=C2B2.offset + OUT_DIM,
                          ap=[[0, 128], [1, OUT_DIM]]))
    c_ones = consts.tile([128, 1], BF16, tag="c_ones")
    nc.vector.memset(c_ones[:], 1.0)
    # packed [ones64;0 | 0;ones64] for z|z2 stats
    c_oz = consts.tile([128, 2], BF16, tag="c_oz")
    nc.vector.memset(c_oz[:], 0.0)
    nc.vector.memset(c_oz[0:64, 0:1], 1.0)
    nc.vector.memset(c_oz[64:128, 1:2], 1.0)

    def transpose(dst_t, src_t, eng):
        eng.dma_start_transpose(
            dst_t[:].rearrange("p (k q) -> p k q", q=128), src_t[:])

    def block_prog(blk):
        """Generator emitting one block's program; yields at stage breaks."""
        rhs28 = pool.tile([28, N], BF16, tag="rhs28", bufs=W)
        nc.sync.dma_start(rhs28[3:28, :], io["rhs_const"])
        nc.sync.dma_start(rhs28[0:3, :], x_t[:, blk * N:(blk + 1) * N])
        sE = pool.tile([128, J * eD], F32, tag="sE", bufs=W)
        nc.vector.memset(sE[:], -10000.0)
        o_fm = []
        yield

        for l in range(L):
            # ---- xl matmul (moving, multi-rhs PSUM accum) + ACT evac ----
            t_xlf = pool.tile([128, N], BF16, tag="t_xlf", bufs=W)
            for c0 in range(0, N, 1024):
                pt = psA.tile([128, 1024], F32, tag="pt_mm")
                npass = 1 if l == 0 else l
                for i in range(npass):
                    lhs = c_W0p if l == 0 else c_GS[(l, i)]
                    rhs_t = rhs28 if l == 0 else o_fm[i]
                    kdim = 28 if l == 0 else 128
                    for s0 in range(0, 1024, 512):
                        nc.tensor.matmul(pt[:, s0:s0 + 512], lhs[0:kdim, :],
                                         rhs_t[0:kdim, c0 + s0:c0 + s0 + 512],
                                         start=(i == 0), stop=(i == npass - 1))
                nc.scalar.activation(t_xlf[:, c0:c0 + 1024], pt[:],
                                     AF.Identity, bias=c_xb[l][:, 0:1],
                                     scale=1.0)
            # ---- scores (micro, stationary=data) -> psum [128, 192] ----
            ps_s = ps_small()
            for j in range(J):
                npass = 1 if l == 0 else l
                for i in range(npass):
                    lhs_t = rhs28 if l == 0 else o_fm[i]
                    kdim = 28 if l == 0 else 128
                    wmat = c_WS0p if l == 0 else c_WSP[(l, i)]
                    nc.tensor.matmul(ps_s[:, j * 8:(j + 1) * 8],
                                     lhs_t[0:kdim, j * jD:(j + 1) * jD],
                                     wmat[0:kdim, :],
                                     start=(i == 0), stop=(i == npass - 1))
            sS = pool.tile([128, J * 8], F32, tag="sS", bufs=W)
            nc.vector.tensor_copy(sS[:], ps_s[:, 0:J * 8])
            yield

            # ---- T1: xl FM -> GM ----
            t_xlg = pool.tile([128, N], BF16, tag="t_xlg", bufs=W)
            transpose(t_xlg, t_xlf, nc.sync)
            if dbg_l == l and "dbg_sS" in io:
                nc.sync.dma_start(io["dbg_sS"], sS[:])

            # ---- E build (gpsimd) ----
            for (src0, sstep, dst0, n, k) in RUNS:
                out_ap = rawap(sE, dst0 * eD + k * HEADS,
                               [(eD, n), (1, HEADS)])
                as_ap = rawap(sS, src0 * 8, [(8 * sstep, n), (1, HEADS)])
                ad_ap = rawap(sS, dst0 * 8 + 4, [(8, n), (1, HEADS)])
                nc.gpsimd.tensor_tensor(out=out_ap, in0=as_ap, in1=ad_ap,
                                        op=ALU.add)
            yield

            # ---- P = exp(lrelu(E)) (V 3-op + S exp); den; alpha ----
            sP = pool.tile([128, J * eD], F32, tag="sP", bufs=W)
            sP2 = pool.tile([128, J * eD], F32, tag="sP2", bufs=1)
            nc.vector.tensor_scalar(sP[:], sE[:], 0.0, SLOPE, op0=ALU.min,
                                    op1=ALU.mult)
            nc.vector.tensor_scalar_max(sP2[:], sE[:], 0.0)
            nc.vector.tensor_tensor(out=sP[:], in0=sP[:], in1=sP2[:],
                                    op=ALU.add)
            nc.scalar.activation(sP[:], sP[:], AF.Exp)
            sden = pool.tile([128, J * HEADS], F32, tag="sden", bufs=W)
            sdr = pool.tile([128, J * HEADS], F32, tag="sdr", bufs=W)
            nc.vector.tensor_reduce(
                out=sden[:].rearrange("p (d h) -> p d h", d=J),
                in_=rawap(sP, 0, [(eD, J), (1, HEADS), (HEADS, KMAX)]),
                axis=AX.X, op=ALU.add)
            nc.vector.reciprocal(sdr[:], sden[:])
            sA = pool.tile([128, J * eD], BF16, tag="sA", bufs=W)
            nc.gpsimd.tensor_tensor(
                out=rawap(sA, 0, [(eD, J), (HEADS, KMAX), (1, HEADS)]),
                in0=rawap(sP, 0, [(eD, J), (HEADS, KMAX), (1, HEADS)]),
                in1=rawap(sdr, 0, [(HEADS, J), (0, KMAX), (1, HEADS)]),
                op=ALU.mult)
            if dbg_l == l and "dbg_sA" in io:
                nc.sync.dma_start(io["dbg_sA"], sA[:])
            if dbg_l == l and "dbg_xlg" in io:
                nc.sync.dma_start(io["dbg_xlg"], t_xlg[:])
            yield

            # ---- aggregation (V) ----
            t_v = pool.tile([128, N], BF16, tag="t_v", bufs=W)

            def xl_ap(j0, sstep, n):
                return rawap(t_xlg, j0 * jD,
                             [(jD * sstep, n), (HEADS, C), (1, HEADS)])

            def al_ap(dst0, n, k):
                return rawap(sA, dst0 * aD + k * HEADS,
                             [(aD, n), (0, C), (1, HEADS)])

            def v_ap(dst0, n, buf):
                return rawap(buf, dst0 * jD, [(jD, n), (HEADS, C), (1, HEADS)])

            for ri, (src0, sstep, dst0, n, k) in enumerate(RUNS):
                if ri == 0:
                    nc.vector.tensor_tensor(out=v_ap(0, 24, t_v),
                                            in0=xl_ap(0, 1, 24),
                                            in1=al_ap(0, 24, 0), op=ALU.mult)
                    continue
                t_tmp = pool.tile([128, N], BF16, tag="t_tmp", bufs=1)
                nc.vector.tensor_tensor(out=v_ap(dst0, n, t_tmp),
                                        in0=xl_ap(src0, sstep, n),
                                        in1=al_ap(dst0, n, k), op=ALU.mult)
                nc.vector.tensor_tensor(out=v_ap(dst0, n, t_v),
                                        in0=v_ap(dst0, n, t_v),
                                        in1=v_ap(dst0, n, t_tmp), op=ALU.add)
            if dbg_l == l and "dbg_v" in io:
                nc.sync.dma_start(io["dbg_v"], t_v[:])
            yield

            # ---- elu: w = max(v, min(exp(v),1)-1) ----
            e32 = pool.tile([128, N], F32, tag="e32", bufs=1)
            nc.scalar.activation(e32[:], t_v[:], AF.Exp)
            t_e1 = pool.tile([128, N], BF16, tag="t_e1", bufs=1)
            nc.vector.tensor_scalar(t_e1[:], e32[:], 1.0, -1.0, op0=ALU.min,
                                    op1=ALU.add)
            t_w = pool.tile([128, N], BF16, tag="t_w", bufs=W)
            nc.vector.tensor_tensor(out=t_w[:], in0=t_v[:], in1=t_e1[:],
                                    op=ALU.max)
            if dbg_l == l and "dbg_w" in io:
                nc.sync.dma_start(io["dbg_w"], t_w[:])
            yield

            # ---- T2 + square (gpsimd) + stats micro-matmuls ----
            t_wf = pool.tile([128, N], BF16, tag="t_wf", bufs=W)
            transpose(t_wf, t_w, nc.scalar)
            t_w2f = pool.tile([128, N], BF16, tag="t_w2f", bufs=1)
            nc.gpsimd.tensor_tensor(out=t_w2f[:], in0=t_wf[:], in1=t_wf[:],
                                    op=ALU.mult)
            ps_t = ps_small()
            for j in range(J):
                nc.tensor.matmul(ps_t[:, j * 2:j * 2 + 1],
                                 t_wf[:, j * jD:(j + 1) * jD],
                                 c_ones[:], start=True, stop=True)
                nc.tensor.matmul(ps_t[:, j * 2 + 1:j * 2 + 2],
                                 t_w2f[:, j * jD:(j + 1) * jD],
                                 c_ones[:], start=True, stop=True)
            sst = pool.tile([128, J * 2], F32, tag="sst", bufs=W)
            nc.vector.tensor_copy(sst[:], ps_t[:, 0:J * 2])
            yield

            # ---- LN smalls: mu, rstd; bf16 x4-replicated ----
            smu = pool.tile([128, J], F32, tag="smu", bufs=1)
            svar = pool.tile([128, J], F32, tag="svar", bufs=1)
            sr = pool.tile([128, J], F32, tag="sr", bufs=1)
            sm2 = pool.tile([128, J], F32, tag="sm2", bufs=1)
            st1 = pool.tile([128, J], F32, tag="st1", bufs=1)
            st2 = pool.tile([128, J], F32, tag="st2", bufs=1)
            stv = sst[:].rearrange("p (m s) -> p m s", s=2)
            nc.vector.tensor_scalar_mul(smu[:], stv[:, :, 0], 1.0 / H)
            nc.vector.tensor_scalar_mul(svar[:], stv[:, :, 1], 1.0 / H)
            nc.vector.tensor_tensor(out=sm2[:], in0=smu[:], in1=smu[:],
                                    op=ALU.mult)
            nc.vector.tensor_tensor(out=svar[:], in0=svar[:], in1=sm2[:],
                                    op=ALU.subtract)
            nc.vector.tensor_scalar_add(svar[:], svar[:], 1e-5)
            emit_rsqrt(nc, sr[:], svar[:], st1[:], st2[:])
            nc.vector.tensor_tensor(out=sm2[:], in0=smu[:], in1=sr[:],
                                    op=ALU.mult)
            # replicate x4 in bf16 for 2x-mode broadcast TTs
            sr4 = pool.tile([128, J * 4], BF16, tag="sr4", bufs=W)
            sm24 = pool.tile([128, J * 4], BF16, tag="sm24", bufs=W)
            nc.vector.tensor_copy(rawap(sr4, 0, [(4, J), (1, 4)]),
                                  rawap(sr, 0, [(1, J), (0, 4)]))
            nc.vector.tensor_copy(rawap(sm24, 0, [(4, J), (1, 4)]),
                                  rawap(sm2, 0, [(1, J), (0, 4)]))

            # ---- LNraw = w*r - mu*r (V, 2x via x4-replicated bf16) ----
            t_lnr = pool.tile([128, N], BF16, tag="t_lnr", bufs=W)
            rb = rawap(sr4, 0, [(4, J), (0, C), (1, HEADS)])
            m2b = rawap(sm24, 0, [(4, J), (0, C), (1, HEADS)])
            v3 = rawap(t_lnr, 0, [(jD, J), (HEADS, C), (1, HEADS)])
            w3 = rawap(t_w, 0, [(jD, J), (HEADS, C), (1, HEADS)])
            nc.vector.tensor_tensor(out=v3, in0=w3, in1=rb, op=ALU.mult)
            nc.vector.tensor_tensor(out=v3, in0=v3, in1=m2b, op=ALU.subtract)
            yield

            # ---- T3: o_l = LNraw in FM ----
            o_l = pool.tile([128, N], BF16, tag=f"o{l}", bufs=W)
            transpose(o_l, t_lnr, nc.scalar)
            o_fm.append(o_l)
            if dbg_l == l and "dbg_h" in io:
                nc.sync.dma_start(io["dbg_h"], o_l[:])
            yield

        # ================= final MLP =================
        t_zz = pool.tile([128, N], BF16, tag="t_zz", bufs=W)
        for c0 in range(0, N, 1024):
            pt = psA.tile([128, 1024], F32, tag="pt_mm")
            for s0 in range(0, 1024, 512):
                for i in range(L):
                    nc.tensor.matmul(pt[:, s0:s0 + 512], c_W1S[i][:],
                                     o_fm[i][:, c0 + s0:c0 + s0 + 512],
                                     start=(i == 0), stop=False)
                nc.tensor.matmul(pt[:, s0:s0 + 512], c_RW1p[:],
                                 rhs28[:, c0 + s0:c0 + s0 + 512],
                                 start=False, stop=True)
            nc.scalar.activation(t_zz[:, c0:c0 + 1024], pt[:],
                                 AF.Relu, bias=c_b1[:, 0:1], scale=1.0)
        nc.gpsimd.tensor_tensor(out=t_zz[64:128, :], in0=t_zz[64:128, :],
                                in1=t_zz[64:128, :], op=ALU.mult)
        yield

        # packed z|z2 stats: one LDW + one FD=2 matmul per joint
        ps_t = ps_small()
        for j in range(J):
            nc.tensor.matmul(ps_t[:, j * 2:j * 2 + 2],
                             t_zz[:, j * jD:(j + 1) * jD],
                             c_oz[:], start=True, stop=True)
        sstz = pool.tile([128, J * 2], F32, tag="sstz", bufs=W)
        nc.vector.tensor_copy(sstz[:], ps_t[:, 0:J * 2])

        smu = pool.tile([128, J], F32, tag="zmu", bufs=1)
        svar = pool.tile([128, J], F32, tag="zvar", bufs=1)
        sr = pool.tile([128, J], F32, tag="zsr", bufs=W)
        sm2 = pool.tile([128, J], F32, tag="zsm2", bufs=W)
        st1 = pool.tile([128, J], F32, tag="zst1", bufs=1)
        st2 = pool.tile([128, J], F32, tag="zst2", bufs=1)
        stv = sstz[:].rearrange("p (m s) -> p m s", s=2)
        nc.vector.tensor_scalar_mul(smu[:], stv[:, :, 0], 2.0 / H)
        nc.vector.tensor_scalar_mul(svar[:], stv[:, :, 1], 2.0 / H)
        nc.vector.tensor_tensor(out=sm2[:], in0=smu[:], in1=smu[:],
                                op=ALU.mult)
        nc.vector.tensor_tensor(out=svar[:], in0=svar[:], in1=sm2[:],
                                op=ALU.subtract)
        nc.vector.tensor_scalar_add(svar[:], svar[:], 1e-5)
        emit_rsqrt(nc, sr[:], svar[:], st1[:], st2[:])
        nc.vector.tensor_tensor(out=sm2[:], in0=smu[:], in1=sr[:],
                                op=ALU.mult)
        yield

        # st6: y-partial = z^T @ W2p per joint -> [128 cols, 6]
        ps_y = ps_small()
        for j in range(J):
            nc.tensor.matmul(ps_y[:, j * OUT_DIM:(j + 1) * OUT_DIM],
                             t_zz[0:64, j * jD:(j + 1) * jD],
                             c_W2p[:], start=True, stop=True)
        st6 = pool.tile([128, J * OUT_DIM], F32, tag="st6", bufs=1)
        nc.vector.tensor_copy(st6[:], ps_y[:, 0:J * OUT_DIM])

        sy = pool.tile([128, J * OUT_DIM], F32, tag="sy", bufs=W)
        t6v = st6[:].rearrange("p (j o) -> p j o", j=J)
        yv = sy[:].rearrange("p (j o) -> p j o", j=J)
        rb = sr[:].unsqueeze(2).broadcast_to((128, J, OUT_DIM))
        m2b = sm2[:].unsqueeze(2).broadcast_to((128, J, OUT_DIM))
        c2b = rawap(c_c2, 0, [(0, J), (1, OUT_DIM)])
        b2b = rawap(c_b2p, 0, [(0, J), (1, OUT_DIM)])
        nc.vector.tensor_tensor(out=yv, in0=t6v, in1=rb, op=ALU.mult)
        nc.vector.tensor_tensor(out=t6v, in0=m2b, in1=c2b, op=ALU.mult)
        nc.vector.tensor_tensor(out=yv, in0=yv, in1=t6v, op=ALU.subtract)
        nc.vector.tensor_tensor(out=yv, in0=yv, in1=b2b, op=ALU.add)

        if y_out is not None:
            yo = bass.AP(
                tensor=y_out.tensor,
                offset=y_out.offset + blk * G * J * OUT_DIM,
                ap=[[J * OUT_DIM, 128], [OUT_DIM, J], [1, OUT_DIM]])
            nc.sync.dma_start(yo, sy[:].rearrange("p (j o) -> p j o", j=J))
        yield

    # ---- software-pipelined emission: W blocks interleaved ----
    for bb in range(0, NB, W):
        gens = [block_prog(bb + w) for w in range(min(W, NB - bb))]
        alive = list(gens)
        while alive:
            nxt = []
            for g in alive:
                try:
                    next(g)
                    nxt.append(g)
                except StopIteration:
                    pass
            alive = nxt


# ======================================================================
# Host driver: kernel(**inputs) -> np.ndarray
# ======================================================================
G2_FULL = 1
NB_FULL = (B // N_CORES) // 128


def _install_ntff_shim():
    import sys, types
    if "antenv.axon_hooks" in sys.modules:
        return
    mod = types.ModuleType("antenv.axon_hooks")
    mod._hook = None
    mod.set_axon_ntff_profile_hook = lambda h: setattr(mod, "_hook", h)
    mod.get_axon_ntff_profile_hook = lambda: mod._hook
    sys.modules["antenv.axon_hooks"] = mod
    try:
        from trn_agent_boot.trn_boot import _ntff_profile_via_ctypes
        mod.set_axon_ntff_profile_hook(
            _ntff_profile_via_ctypes("/opt/axon/libaxon_pjrt.so"))
    except Exception:
        pass
    try:
        import concourse.bass_utils as bu
        bu.upload_artifacts = lambda tmpdir: tmpdir
    except Exception:
        pass


_NC_CACHE = {}


def _build_nc(NB):
    key = NB
    if key in _NC_CACHE:
        return _NC_CACHE[key]
    from contextlib import ExitStack
    from concourse import bacc
    nc = bacc.Bacc("TRN2", target_bir_lowering=False, debug=False,
                   num_devices=N_CORES)
    G = 128
    BCk = G * NB
    dt = nc.dram_tensor
    io = {}
    def din(name, shape, dtype):
        io[name] = dt(name, shape, dtype, kind="ExternalInput").ap()
    din("x_t", (IN_DIM, BCk * J), BF16)
    din("rhs_const", (25, J * G), BF16)
    din("W0p", (28, H), BF16)
    din("WS0p", (28, 8), BF16)
    din("GS", (L, L, H, H), BF16)
    din("WSP", (L, L, H, 8), BF16)
    din("XBIAS", (L, H), F32)
    din("W1S", (L, H, H), BF16)
    din("RW1p", (28, H), BF16)
    din("B1", (H,), F32)
    din("W2p", (H // 2, OUT_DIM), BF16)
    din("C2B2", (2, OUT_DIM), F32)
    io["y"] = dt("y", (BCk, J, OUT_DIM), F32, kind="ExternalOutput").ap()
    with tile.TileContext(nc) as tc:
        with ExitStack() as ctx:
            kernel_body(ctx, tc, io, NB)
    nc.compile()
    _NC_CACHE[key] = nc
    return nc


def make_in_maps(inputs, NB=NB_FULL):
    import ml_dtypes
    bf = ml_dtypes.bfloat16
    pp = host_prep(inputs)
    G = 128
    BCk = G * NB
    x = np.asarray(inputs["x"], dtype=np.float32)
    shared = dict(
        rhs_const=np.ascontiguousarray(make_rhs_const(G)),
        W0p=np.ascontiguousarray(pp["W0p"].astype(bf)),
        WS0p=np.ascontiguousarray(pp["WS0p"].astype(bf)),
        GS=np.ascontiguousarray(pp["GS"].astype(bf)),
        WSP=np.ascontiguousarray(pp["WSP"].astype(bf)),
        XBIAS=np.ascontiguousarray(pp["XBIAS"].astype(np.float32)),
        W1S=np.ascontiguousarray(pp["W1S"].astype(bf)),
        RW1p=np.ascontiguousarray(pp["RW1p"].astype(bf)),
        B1=np.ascontiguousarray(pp["b1"].astype(np.float32)),
        W2p=np.ascontiguousarray(pp["W2p"].astype(bf)),
        C2B2=np.ascontiguousarray(
            np.stack([pp["c2"], pp["b2p"]]).astype(np.float32)),
    )
    in_maps = []
    for core in range(N_CORES):
        xc = x[core * BCk:(core + 1) * BCk]
        m = dict(shared)
        m["x_t"] = np.ascontiguousarray(make_x_t(xc, G))
        in_maps.append(m)
    return in_maps


def run_on_cores(inputs, NB=NB_FULL, trace=False):
    _install_ntff_shim()
    from concourse.bass_utils import run_bass_kernel_spmd
    nc = _build_nc(NB)
    in_maps = make_in_maps(inputs, NB)
    res = run_bass_kernel_spmd(nc, in_maps, list(range(N_CORES)), trace=trace)
    ys = [res.results[c]["y"] for c in range(N_CORES)]
    y = np.concatenate(ys, axis=0).astype(np.float32)
    return y, res


def kernel(**inputs) -> np.ndarray:
    y, _ = run_on_cores(inputs)
    return y


# revision 20
# speedup vs baseline: 2.0928x; 1.0977x over previous
"""Self-contained TRN2 Bass kernel for nn_GATRotationRegressor.

kernel(**inputs) -> [16384, 24, 6] fp32. Data-parallel over 8 NeuronCores;
all layouts/shapes hardcoded for B=16384, J=24, H=128, heads=4, L=3.

v2: software-pipelined (W blocks interleaved), PSUM residual folding,
fused elu, LN gamma/beta folded into downstream matmuls, engine rebalance.
"""
from contextlib import ExitStack

import numpy as np

import concourse.bass as bass
import concourse.tile as tile
from concourse import mybir


PARENTS = [-1, 0, 0, 0, 1, 2, 3, 4, 5, 6, 7, 8, 9, 9, 9, 12, 13, 14, 16, 17, 18, 19, 20, 21]
B, J, IN_DIM, H, HEADS, OUT_DIM, L = 16384, 24, 3, 128, 4, 6, 3
C = H // HEADS
SLOPE = 0.2
KMAX = 5           # padded neighbor slots per dst
# feature permutation: device feature n = c*4 + h  <->  model feature o = h*32 + c
FPERM_O_OF_N = np.array([(n % HEADS) * C + n // HEADS for n in range(H)])
N_CORES = 8
BC = B // N_CORES  # graphs per core


def children(j):
    return [c for c, p in enumerate(PARENTS) if p == j]


def edge_slots():
    """For every real edge (src, dst) return its slot k at dst."""
    slots = {}
    for j in range(J):
        slots[(j, j)] = 0
        p = PARENTS[j]
        if p >= 0:
            slots[(p, j)] = 1
        for i, c in enumerate(children(j)):
            slots[(c, j)] = 2 + i
    return slots


def build_runs():
    """Greedy decomposition of the 70 edges into strided runs.

    Returns list of (src0, sstep, dst0, n, slot)."""
    slots = edge_slots()
    edges = sorted(slots.keys(), key=lambda e: (slots[e], e[1]))
    runs = []
    used = set()
    for e in edges:
        if e in used:
            continue
        src, dst = e
        k = slots[e]
        for sstep in (1, 0):
            n = 1
            while True:
                nxt = (src + sstep * n, dst + n)
                if nxt in slots and slots[nxt] == k and nxt not in used:
                    n += 1
                else:
                    break
            if n > 1 or sstep == 0:
                break
        for i in range(n):
            used.add((src + sstep * i, dst + i))
        runs.append((src, sstep, dst, n, k))
    assert sum(r[3] for r in runs) == 70, sum(r[3] for r in runs)
    return runs


def _edges():
    e = []
    for c, p in enumerate(PARENTS):
        if p >= 0:
            e.append((p, c)); e.append((c, p))
    for j in range(len(PARENTS)):
        e.append((j, j))
    a = np.asarray(e, dtype=np.int32)
    return a[:, 0], a[:, 1]


def _ln_np(x, g, b, eps=1e-5):
    m = x.mean(-1, keepdims=True)
    v = x.var(-1, keepdims=True)
    return (x - m) / np.sqrt(v + eps) * g + b


def np_reference(x, p, collect=None):
    """Numpy port of reference.py for an arbitrary batch."""
    Bn = x.shape[0]
    src, dst = _edges()
    h = x @ p["in_w"] + p["in_b"] + p["pos"][None]
    res = x @ p["res_w"] + p["res_b"]
    cc = collect if collect is not None else {}
    for l in range(L):
        hp = h
        xl = (h @ p["gat_w"][l]).reshape(Bn, J, HEADS, C)
        a_s = np.einsum('bjhc,hc->bjh', xl, p["att_s"][l])
        a_d = np.einsum('bjhc,hc->bjh', xl, p["att_d"][l])
        cc[f"xl{l}"] = xl; cc[f"a_s{l}"] = a_s; cc[f"a_d{l}"] = a_d
        e = a_s[:, src] + a_d[:, dst]
        e = np.where(e > 0, e, SLOPE * e)
        ex = np.exp(e)
        den = np.zeros((Bn, J, HEADS), e.dtype)
        np.add.at(den, (slice(None), dst), ex)
        alpha = ex / den[:, dst]
        cc[f"alpha{l}"] = alpha
        msg = xl[:, src] * alpha[..., None]
        out = np.zeros_like(xl)
        np.add.at(out, (slice(None), dst), msg)
        out = out.reshape(Bn, J, H) + p["gat_b"][l]
        cc[f"agg{l}"] = out
        out = np.where(out > 0, out, np.exp(np.minimum(out, 0)) - 1)  # elu
        out = _ln_np(out, p["ln_g"][l], p["ln_b"][l])
        cc[f"o{l}"] = out
        h = out + hp if l > 0 else out
    h = h + res
    y = np.maximum(h @ p["w1"] + p["b1"], 0)
    y = _ln_np(y, p["lng2"], p["lnb2"])
    return y @ p["w2"] + p["b2"]


def host_prep(inputs):
    """Host-side weight algebra: per-pass folded stationaries + biases."""
    f32 = np.float32
    in_w, in_b = np.asarray(inputs["in_w"]), np.asarray(inputs["in_b"])
    res_w, res_b = np.asarray(inputs["res_w"]), np.asarray(inputs["res_b"])
    pos = np.asarray(inputs["pos"])
    gat_w = np.asarray(inputs["gat_w"])
    att_s, att_d = np.asarray(inputs["att_s"]), np.asarray(inputs["att_d"])
    gat_b = np.asarray(inputs["gat_b"])
    ln_g, ln_b = np.asarray(inputs["ln_g"]), np.asarray(inputs["ln_b"])
    w1, b1 = np.asarray(inputs["w1"]), np.asarray(inputs["b1"])
    lng2, lnb2 = np.asarray(inputs["lng2"]), np.asarray(inputs["lnb2"])
    w2, b2 = np.asarray(inputs["w2"]), np.asarray(inputs["b2"])

    # per-layer score projections: ws[l] [H, 8] cols = (a_s h0..h3, a_d h0..h3)
    ws = np.zeros((L, H, 8), f32)
    for l in range(L):
        for h in range(HEADS):
            ws[l, :, h] = gat_w[l, :, h * C:(h + 1) * C] @ att_s[l, h]
            ws[l, :, 4 + h] = gat_w[l, :, h * C:(h + 1) * C] @ att_d[l, h]

    P = FPERM_O_OF_N

    # L0 folded stationaries on rhs28 = [x(3); ones(1); onehot_j(24)]
    W0p = np.zeros((28, H), f32)
    W0p[0:3] = in_w @ gat_w[0]
    W0p[3] = in_b @ gat_w[0]
    W0p[4:28] = pos @ gat_w[0]
    W0p = W0p[:, P]
    WS0p = np.zeros((28, 8), f32)
    WS0p[0:3] = in_w @ ws[0]
    WS0p[3] = in_b @ ws[0]
    WS0p[4:28] = pos @ ws[0]

    gat_w_d = gat_w[:, P][:, :, P]          # rows+cols permuted
    ws_d = ws[:, P]                          # rows permuted
    gat_b_d = gat_b[:, P]
    ln_g_d = ln_g[:, P]
    ln_b_d = ln_b[:, P]
    w1_d = w1[P, :]

    # Per-(layer, pass) stationaries with LN gamma folded in.
    # h_l = sum_{i<l} (g_i*o_i + lnb_i)   (o_i = LN-raw of layer i)
    # xl_l = gat_w_l^T h_l + (bias: gat_b_l + gat_w_l^T sum lnb_i)
    GS = np.zeros((L, L, H, H), f32)    # GS[l][i], valid i<l, l>=1
    WSP = np.zeros((L, L, H, 8), f32)
    XBIAS = np.zeros((L, H), f32)
    SBIAS = np.zeros((L, 8), f32)
    for l in range(L):
        lnb_sum = ln_b_d[:l].sum(axis=0) if l > 0 else np.zeros(H, f32)
        XBIAS[l] = gat_b_d[l] + lnb_sum @ gat_w_d[l]
        SBIAS[l] = lnb_sum @ ws_d[l]
        for i in range(l):
            GS[l, i] = ln_g_d[i][:, None] * gat_w_d[l]
            WSP[l, i] = ln_g_d[i][:, None] * ws_d[l]
    # score bias must be zero for E-build simplification (true: ln_b == 0)
    assert np.abs(SBIAS).max() == 0.0, "nonzero score bias not supported"

    # w1 consumed h_L + res; h_L = sum_i (g_i o_i + lnb_i)
    # doubled columns: psum rows 0:64 = z, rows 64:128 = z copy (squared at evac)
    W1S = np.zeros((L, H, H), f32)
    for i in range(L):
        w1s = ln_g_d[i][:, None] * w1_d
        W1S[i][:, 0:64] = w1s
        W1S[i][:, 64:128] = w1s
    lnb_sum = ln_b_d.sum(axis=0)
    RW1p = np.zeros((28, H), f32)
    rw = res_w @ w1
    RW1p[0:3, 0:64] = rw
    RW1p[0:3, 64:128] = rw
    rb1 = res_b @ w1 + lnb_sum @ w1_d        # b1 added at evac via ACT bias
    RW1p[3, 0:64] = rb1
    RW1p[3, 64:128] = rb1
    b1d = np.concatenate([b1, b1])
    # final LN2/w2 fold
    W2p = (lng2[:, None] * w2).astype(f32)          # [64, 6]
    c2 = W2p.sum(axis=0)                             # colsum for -mu*r term
    b2p = lnb2 @ w2 + b2                             # [6]

    return dict(
        W0p=W0p, WS0p=WS0p, GS=GS, WSP=WSP, XBIAS=XBIAS,
        W1S=W1S, RW1p=RW1p, W2p=W2p, c2=c2, b2p=b2p,
        ln_g=ln_g_d, ln_b=ln_b_d, b1=b1d,
    )


def make_rhs_const(G):
    """Rows 3..27 of rhs28: [ones; onehot_j] as [25, J*G] bf16."""
    import ml_dtypes
    N = J * G
    out = np.zeros((25, N), np.float32)
    out[0] = 1.0
    for j in range(J):
        out[1 + j, j * G:(j + 1) * G] = 1.0
    return out.astype(ml_dtypes.bfloat16)


def make_x_t(x_core, G):
    """x_core [BCk, 24, 3] -> x_t [3, BCk*24] bf16 with col = blk*G*24 + j*G + g."""
    import ml_dtypes
    BCk = x_core.shape[0]
    nblk = BCk // G
    xt = x_core.reshape(nblk, G, J, IN_DIM).transpose(3, 0, 2, 1).reshape(IN_DIM, BCk * J)
    return xt.astype(ml_dtypes.bfloat16)


F32 = mybir.dt.float32
BF16 = mybir.dt.bfloat16
AF = mybir.ActivationFunctionType
ALU = mybir.AluOpType
AX = mybir.AxisListType

RUNS = build_runs()
W = 2                 # software pipeline width (blocks in flight)


def rawap(t, off, dims):
    a = t[:]
    return bass.AP(tensor=a.tensor, offset=a.offset + off,
                   ap=[[a.ap[0][0], a.ap[0][1]]] + [list(d) for d in dims])


def emit_rsqrt(nc, out, in_, tmp, tmp2):
    """out = 1/sqrt(in_) via quake init + 2 Newton iters. All [128, F] F32."""
    I32 = mybir.dt.int32
    ib = in_.bitcast(I32)
    nc.vector.tensor_scalar(tmp.bitcast(I32), ib, 1, None,
                            op0=ALU.logical_shift_right)
    nc.vector.tensor_scalar(tmp.bitcast(I32), tmp.bitcast(I32), -1, 0x5F3759DF,
                            op0=ALU.mult, op1=ALU.add)
    for _ in range(2):
        nc.vector.tensor_tensor(out=tmp2, in0=tmp, in1=tmp, op=ALU.mult)
        nc.vector.tensor_tensor(out=tmp2, in0=tmp2, in1=in_, op=ALU.mult)
        nc.vector.tensor_scalar(tmp2, tmp2, -0.5, 1.5, op0=ALU.mult, op1=ALU.add)
        nc.vector.tensor_tensor(out=tmp, in0=tmp, in1=tmp2, op=ALU.mult)
    nc.vector.tensor_copy(out, tmp)


def kernel_body(ctx, tc, io, n_blocks, dbg_l=None):
    """io: dict name -> bass.AP (dram). Emits the kernel. G2=1 layout."""
    nc = tc.nc
    G = 128
    N = J * G            # 3072
    NB = n_blocks
    jD = 128             # per-joint column span
    aD = HEADS * KMAX    # 20: per-dst slot span in sE/sA
    eD = aD

    x_t, y_out = io["x_t"], io.get("y")

    pool = ctx.enter_context(tc.tile_pool(name="pool", bufs=1))
    consts = ctx.enter_context(tc.tile_pool(name="consts", bufs=1))
    psA = ctx.enter_context(tc.tile_pool(name="psA", bufs=2, space="PSUM"))
    psB = ctx.enter_context(tc.tile_pool(name="psB", bufs=3, space="PSUM"))

    def ps_small():
        return psB.tile([128, 512], F32, tag="ps_small", name="ps_small")

    # ---- persistent constants ----
    def cload(name, shape, dtype=BF16, src=None):
        t = consts.tile(list(shape), dtype, tag=f"c_{name}")
        nc.sync.dma_start(t[:], src if src is not None else io[name])
        return t

    c_W0p = cload("W0p", (28, H))
    c_WS0p = cload("WS0p", (28, 8))
    c_GS = {}
    c_WSP = {}
    for l in (1, 2):
        for i in range(l):
            c_GS[(l, i)] = cload(f"GS{l}{i}", (H, H), src=io["GS"][l][i])
            c_WSP[(l, i)] = cload(f"WSP{l}{i}", (H, 8), src=io["WSP"][l][i])
    c_W1S = [cload(f"W1S{i}", (H, H), src=io["W1S"][i]) for i in range(L)]
    c_RW1p = cload("RW1p", (28, H))
    c_W2p = cload("W2p", (H // 2, OUT_DIM))
    c_xb = [cload(f"xb{l}", (H, 1), F32, io["XBIAS"][l].unsqueeze(1))
            for l in range(L)]
    c_b1 = cload("b1", (H, 1), F32, io["B1"].unsqueeze(1))
    C2B2 = io["C2B2"]
    c_c2 = cload("c2", (128, OUT_DIM), F32,
                 bass.AP(tensor=C2B2.tensor, offset=C2B2.offset,
                         ap=[[0, 128], [1, OUT_DIM]]))
    c_b2p = cload("b2p", (128, OUT_DIM), F32,
                  bass.AP(tensor=C2B2.tensor, offset=C2B2.offset + OUT_DIM,
                          ap=[[0, 128], [1, OUT_DIM]]))
    c_ones = consts.tile([128, 1], BF16, tag="c_ones")
    nc.vector.memset(c_ones[:], 1.0)
    # packed [ones64;0 | 0;ones64] for z|z2 stats
    c_oz = consts.tile([128, 2], BF16, tag="c_oz")
    nc.vector.memset(c_oz[:], 0.0)
    nc.vector.memset(c_oz[0:64, 0:1], 1.0)
    nc.vector.memset(c_oz[64:128, 1:2], 1.0)

    def transpose(dst_t, src_t, eng):
        eng.dma_start_transpose(
            dst_t[:].rearrange("p (k q) -> p k q", q=128), src_t[:])

    def block_prog(blk):
        """Generator emitting one block's program; yields at stage breaks."""
        rhs28 = pool.tile([28, N], BF16, tag="rhs28", bufs=W)
        nc.sync.dma_start(rhs28[3:28, :], io["rhs_const"])
        nc.sync.dma_start(rhs28[0:3, :], x_t[:, blk * N:(blk + 1) * N])
        sE = pool.tile([128, J * eD], F32, tag="sE", bufs=W)
        nc.vector.memset(sE[:], -10000.0)
        o_fm = []
        yield

        for l in range(L):
            # ---- xl matmul (moving, multi-rhs PSUM accum) + ACT evac ----
            t_xlf = pool.tile([128, N], BF16, tag="t_xlf", bufs=W)
            for c0 in range(0, N, 1024):
                pt = psA.tile([128, 1024], F32, tag="pt_mm")
                npass = 1 if l == 0 else l
                for i in range(npass):
                    lhs = c_W0p if l == 0 else c_GS[(l, i)]
                    rhs_t = rhs28 if l == 0 else o_fm[i]
                    kdim = 28 if l == 0 else 128
                    for s0 in range(0, 1024, 512):
                        nc.tensor.matmul(pt[:, s0:s0 + 512], lhs[0:kdim, :],
                                         rhs_t[0:kdim, c0 + s0:c0 + s0 + 512],
                                         start=(i == 0), stop=(i == npass - 1))
                nc.scalar.activation(t_xlf[:, c0:c0 + 1024], pt[:],
                                     AF.Identity, bias=c_xb[l][:, 0:1],
                                     scale=1.0)
            # ---- scores (micro, stationary=data) -> psum [128, 192] ----
            ps_s = ps_small()
            for j in range(J):
                npass = 1 if l == 0 else l
                for i in range(npass):
                    lhs_t = rhs28 if l == 0 else o_fm[i]
                    kdim = 28 if l == 0 else 128
                    wmat = c_WS0p if l == 0 else c_WSP[(l, i)]
                    nc.tensor.matmul(ps_s[:, j * 8:(j + 1) * 8],
                                     lhs_t[0:kdim, j * jD:(j + 1) * jD],
                                     wmat[0:kdim, :],
                                     start=(i == 0), stop=(i == npass - 1))
            sS = pool.tile([128, J * 8], F32, tag="sS", bufs=W)
            nc.vector.tensor_copy(sS[:], ps_s[:, 0:J * 8])
            yield

            # ---- T1: xl FM -> GM ----
            t_xlg = pool.tile([128, N], BF16, tag="t_xlg", bufs=W)
            transpose(t_xlg, t_xlf, nc.sync)
            if dbg_l == l and "dbg_sS" in io:
                nc.sync.dma_start(io["dbg_sS"], sS[:])

            # ---- E build (gpsimd) ----
            for (src0, sstep, dst0, n, k) in RUNS:
                out_ap = rawap(sE, dst0 * eD + k * HEADS,
                               [(eD, n), (1, HEADS)])
                as_ap = rawap(sS, src0 * 8, [(8 * sstep, n), (1, HEADS)])
                ad_ap = rawap(sS, dst0 * 8 + 4, [(8, n), (1, HEADS)])
                nc.gpsimd.tensor_tensor(out=out_ap, in0=as_ap, in1=ad_ap,
                                        op=ALU.add)
            yield

            # ---- P = exp(lrelu(E)) (V 3-op + S exp); den; alpha ----
            sP = pool.tile([128, J * eD], F32, tag="sP", bufs=W)
            sP2 = pool.tile([128, J * eD], F32, tag="sP2", bufs=1)
            nc.vector.tensor_scalar(sP[:], sE[:], 0.0, SLOPE, op0=ALU.min,
                                    op1=ALU.mult)
            nc.vector.tensor_scalar_max(sP2[:], sE[:], 0.0)
            nc.vector.tensor_tensor(out=sP[:], in0=sP[:], in1=sP2[:],
                                    op=ALU.add)
            nc.scalar.activation(sP[:], sP[:], AF.Exp)
            sden = pool.tile([128, J * HEADS], F32, tag="sden", bufs=W)
            sdr = pool.tile([128, J * HEADS], F32, tag="sdr", bufs=W)
            nc.vector.tensor_reduce(
                out=sden[:].rearrange("p (d h) -> p d h", d=J),
                in_=rawap(sP, 0, [(eD, J), (1, HEADS), (HEADS, KMAX)]),
                axis=AX.X, op=ALU.add)
            nc.vector.reciprocal(sdr[:], sden[:])
            sA = pool.tile([128, J * eD], BF16, tag="sA", bufs=W)
            nc.gpsimd.tensor_tensor(
                out=rawap(sA, 0, [(eD, J), (HEADS, KMAX), (1, HEADS)]),
                in0=rawap(sP, 0, [(eD, J), (HEADS, KMAX), (1, HEADS)]),
                in1=rawap(sdr, 0, [(HEADS, J), (0, KMAX), (1, HEADS)]),
                op=ALU.mult)
            if dbg_l == l and "dbg_sA" in io:
                nc.sync.dma_start(io["dbg_sA"], sA[:])
            if dbg_l == l and "dbg_xlg" in io:
                nc.sync.dma_start(io["dbg_xlg"], t_xlg[:])
            yield

            # ---- aggregation (V) ----
            t_v = pool.tile([128, N], BF16, tag="t_v", bufs=W)

            def xl_ap(j0, sstep, n):
                return rawap(t_xlg, j0 * jD,
                             [(jD * sstep, n), (HEADS, C), (1, HEADS)])

            def al_ap(dst0, n, k):
                return rawap(sA, dst0 * aD + k * HEADS,
                             [(aD, n), (0, C), (1, HEADS)])

            def v_ap(dst0, n, buf):
                return rawap(buf, dst0 * jD, [(jD, n), (HEADS, C), (1, HEADS)])

            t_tmp = pool.tile([128, N], BF16, tag="t_tmp", bufs=1)
            # mults per run; adds merged over contiguous dst ranges per slot
            pend = []          # (dst0, n) pending adds for current slot
            cur_slot = None

            def flush_adds():
                merged = []
                for d0, nn in sorted(pend):
                    if merged and merged[-1][0] + merged[-1][1] == d0:
                        merged[-1][1] += nn
                    else:
                        merged.append([d0, nn])
                for d0, nn in merged:
                    nc.vector.tensor_tensor(out=v_ap(d0, nn, t_v),
                                            in0=v_ap(d0, nn, t_v),
                                            in1=v_ap(d0, nn, t_tmp),
                                            op=ALU.add)
                pend.clear()

            for ri, (src0, sstep, dst0, n, k) in enumerate(RUNS):
                if ri == 0:
                    nc.vector.tensor_tensor(out=v_ap(0, 24, t_v),
                                            in0=xl_ap(0, 1, 24),
                                            in1=al_ap(0, 24, 0), op=ALU.mult)
                    continue
                if cur_slot is not None and k != cur_slot:
                    flush_adds()
                cur_slot = k
                nc.vector.tensor_tensor(out=v_ap(dst0, n, t_tmp),
                                        in0=xl_ap(src0, sstep, n),
                                        in1=al_ap(dst0, n, k), op=ALU.mult)
                pend.append((dst0, n))
            flush_adds()
            if dbg_l == l and "dbg_v" in io:
                nc.sync.dma_start(io["dbg_v"], t_v[:])
            yield

            # ---- elu: w = max(v, min(exp(v),1)-1) ----
            e32 = pool.tile([128, N], F32, tag="e32", bufs=1)
            nc.scalar.activation(e32[:], t_v[:], AF.Exp)
            t_e1 = pool.tile([128, N], BF16, tag="t_e1", bufs=1)
            nc.vector.tensor_scalar(t_e1[:], e32[:], 1.0, -1.0, op0=ALU.min,
                                    op1=ALU.add)
            t_w = pool.tile([128, N], BF16, tag="t_w", bufs=W)
            nc.vector.tensor_tensor(out=t_w[:], in0=t_v[:], in1=t_e1[:],
                                    op=ALU.max)
            if dbg_l == l and "dbg_w" in io:
                nc.sync.dma_start(io["dbg_w"], t_w[:])
            yield

            # ---- T2 + square (gpsimd) + stats micro-matmuls ----
            t_wf = pool.tile([128, N], BF16, tag="t_wf", bufs=W)
            transpose(t_wf, t_w, nc.scalar)
            t_w2f = pool.tile([128, N], BF16, tag="t_w2f", bufs=1)
            nc.gpsimd.tensor_tensor(out=t_w2f[:], in0=t_wf[:], in1=t_wf[:],
                                    op=ALU.mult)
            ps_t = ps_small()
            for j in range(J):
                nc.tensor.matmul(ps_t[:, j * 2:j * 2 + 1],
                                 t_wf[:, j * jD:(j + 1) * jD],
                                 c_ones[:], start=True, stop=True)
                nc.tensor.matmul(ps_t[:, j * 2 + 1:j * 2 + 2],
                                 t_w2f[:, j * jD:(j + 1) * jD],
                                 c_ones[:], start=True, stop=True)
            sst = pool.tile([128, J * 2], F32, tag="sst", bufs=W)
            nc.vector.tensor_copy(sst[:], ps_t[:, 0:J * 2])
            yield

            # ---- LN smalls: mu, rstd; bf16 x4-replicated ----
            smu = pool.tile([128, J], F32, tag="smu", bufs=1)
            svar = pool.tile([128, J], F32, tag="svar", bufs=1)
            sr = pool.tile([128, J], F32, tag="sr", bufs=1)
            sm2 = pool.tile([128, J], F32, tag="sm2", bufs=1)
            st1 = pool.tile([128, J], F32, tag="st1", bufs=1)
            st2 = pool.tile([128, J], F32, tag="st2", bufs=1)
            stv = sst[:].rearrange("p (m s) -> p m s", s=2)
            nc.vector.tensor_scalar_mul(smu[:], stv[:, :, 0], 1.0 / H)
            nc.vector.tensor_scalar_mul(svar[:], stv[:, :, 1], 1.0 / H)
            nc.vector.tensor_tensor(out=sm2[:], in0=smu[:], in1=smu[:],
                                    op=ALU.mult)
            nc.vector.tensor_tensor(out=svar[:], in0=svar[:], in1=sm2[:],
                                    op=ALU.subtract)
            nc.vector.tensor_scalar_add(svar[:], svar[:], 1e-5)
            emit_rsqrt(nc, sr[:], svar[:], st1[:], st2[:])
            nc.vector.tensor_tensor(out=sm2[:], in0=smu[:], in1=sr[:],
                                    op=ALU.mult)
            # replicate x4 in bf16 for 2x-mode broadcast TTs
            sr4 = pool.tile([128, J * 4], BF16, tag="sr4", bufs=W)
            sm24 = pool.tile([128, J * 4], BF16, tag="sm24", bufs=W)
            nc.vector.tensor_copy(rawap(sr4, 0, [(4, J), (1, 4)]),
                                  rawap(sr, 0, [(1, J), (0, 4)]))
            nc.vector.tensor_copy(rawap(sm24, 0, [(4, J), (1, 4)]),
                                  rawap(sm2, 0, [(1, J), (0, 4)]))

            # ---- LNraw = w*r - mu*r (V, 2x via x4-replicated bf16) ----
            t_lnr = pool.tile([128, N], BF16, tag="t_lnr", bufs=W)
            rb = rawap(sr4, 0, [(4, J), (0, C), (1, HEADS)])
            m2b = rawap(sm24, 0, [(4, J), (0, C), (1, HEADS)])
            v3 = rawap(t_lnr, 0, [(jD, J), (HEADS, C), (1, HEADS)])
            w3 = rawap(t_w, 0, [(jD, J), (HEADS, C), (1, HEADS)])
            nc.vector.tensor_tensor(out=v3, in0=w3, in1=rb, op=ALU.mult)
            nc.vector.tensor_tensor(out=v3, in0=v3, in1=m2b, op=ALU.subtract)
            yield

            # ---- T3: o_l = LNraw in FM ----
            o_l = pool.tile([128, N], BF16, tag=f"o{l}", bufs=W)
            transpose(o_l, t_lnr, nc.sync)
            o_fm.append(o_l)
            if dbg_l == l and "dbg_h" in io:
                nc.sync.dma_start(io["dbg_h"], o_l[:])
            yield

        # ================= final MLP =================
        t_zz = pool.tile([128, N], BF16, tag="t_zz", bufs=W)
        for c0 in range(0, N, 1024):
            pt = psA.tile([128, 1024], F32, tag="pt_mm")
            for s0 in range(0, 1024, 512):
                for i in range(L):
                    nc.tensor.matmul(pt[:, s0:s0 + 512], c_W1S[i][:],
                                     o_fm[i][:, c0 + s0:c0 + s0 + 512],
                                     start=(i == 0), stop=False)
                nc.tensor.matmul(pt[:, s0:s0 + 512], c_RW1p[:],
                                 rhs28[:, c0 + s0:c0 + s0 + 512],
                                 start=False, stop=True)
            nc.scalar.activation(t_zz[:, c0:c0 + 1024], pt[:],
                                 AF.Relu, bias=c_b1[:, 0:1], scale=1.0)
        nc.gpsimd.tensor_tensor(out=t_zz[64:128, :], in0=t_zz[64:128, :],
                                in1=t_zz[64:128, :], op=ALU.mult)
        yield

        # packed z|z2 stats: one LDW + one FD=2 matmul per joint
        ps_t = ps_small()
        for j in range(J):
            nc.tensor.matmul(ps_t[:, j * 2:j * 2 + 2],
                             t_zz[:, j * jD:(j + 1) * jD],
                             c_oz[:], start=True, stop=True)
        sstz = pool.tile([128, J * 2], F32, tag="sstz", bufs=W)
        nc.vector.tensor_copy(sstz[:], ps_t[:, 0:J * 2])

        smu = pool.tile([128, J], F32, tag="zmu", bufs=1)
        svar = pool.tile([128, J], F32, tag="zvar", bufs=1)
        sr = pool.tile([128, J], F32, tag="zsr", bufs=W)
        sm2 = pool.tile([128, J], F32, tag="zsm2", bufs=W)
        st1 = pool.tile([128, J], F32, tag="zst1", bufs=1)
        st2 = pool.tile([128, J], F32, tag="zst2", bufs=1)
        stv = sstz[:].rearrange("p (m s) -> p m s", s=2)
        nc.vector.tensor_scalar_mul(smu[:], stv[:, :, 0], 2.0 / H)
        nc.vector.tensor_scalar_mul(svar[:], stv[:, :, 1], 2.0 / H)
        nc.vector.tensor_tensor(out=sm2[:], in0=smu[:], in1=smu[:],
                                op=ALU.mult)
        nc.vector.tensor_tensor(out=svar[:], in0=svar[:], in1=sm2[:],
                                op=ALU.subtract)
        nc.vector.tensor_scalar_add(svar[:], svar[:], 1e-5)
        emit_rsqrt(nc, sr[:], svar[:], st1[:], st2[:])
        nc.vector.tensor_tensor(out=sm2[:], in0=smu[:], in1=sr[:],
                                op=ALU.mult)
        yield

        # st6: y-partial = z^T @ W2p per joint -> [128 cols, 6]
        ps_y = ps_small()
        for j in range(J):
            nc.tensor.matmul(ps_y[:, j * OUT_DIM:(j + 1) * OUT_DIM],
                             t_zz[0:64, j * jD:(j + 1) * jD],
                             c_W2p[:], start=True, stop=True)
        st6 = pool.tile([128, J * OUT_DIM], F32, tag="st6", bufs=1)
        nc.vector.tensor_copy(st6[:], ps_y[:, 0:J * OUT_DIM])

        sy = pool.tile([128, J * OUT_DIM], F32, tag="sy", bufs=W)
        t6v = st6[:].rearrange("p (j o) -> p j o", j=J)
        yv = sy[:].rearrange("p (j o) -> p j o", j=J)
        rb = sr[:].unsqueeze(2).broadcast_to((128, J, OUT_DIM))
        m2b = sm2[:].unsqueeze(2).broadcast_to((128, J, OUT_DIM))
        c2b = rawap(c_c2, 0, [(0, J), (1, OUT_DIM)])
        b2b = rawap(c_b2p, 0, [(0, J), (1, OUT_DIM)])
        nc.vector.tensor_tensor(out=yv, in0=t6v, in1=rb, op=ALU.mult)
        nc.vector.tensor_tensor(out=t6v, in0=m2b, in1=c2b, op=ALU.mult)
        nc.vector.tensor_tensor(out=yv, in0=yv, in1=t6v, op=ALU.subtract)
        nc.vector.tensor_tensor(out=yv, in0=yv, in1=b2b, op=ALU.add)

        if y_out is not None:
            yo = bass.AP(
                tensor=y_out.tensor,
                offset=y_out.offset + blk * G * J * OUT_DIM,
                ap=[[J * OUT_DIM, 128], [OUT_DIM, J], [1, OUT_DIM]])
            nc.sync.dma_start(yo, sy[:].rearrange("p (j o) -> p j o", j=J))
        yield

    # ---- software-pipelined emission: W blocks interleaved ----
    for bb in range(0, NB, W):
        gens = [block_prog(bb + w) for w in range(min(W, NB - bb))]
        alive = list(gens)
        while alive:
            nxt = []
            for g in alive:
                try:
                    next(g)
                    nxt.append(g)
                except StopIteration:
                    pass
            alive = nxt


# ======================================================================
# Host driver: kernel(**inputs) -> np.ndarray
# ======================================================================
G2_FULL = 1
NB_FULL = (B // N_CORES) // 128


def _install_ntff_shim():
    import sys, types
    if "antenv.axon_hooks" in sys.modules:
        return
    mod = types.ModuleType("antenv.axon_hooks")
    mod._hook = None
    mod.set_axon_ntff_profile_hook = lambda h: setattr(mod, "_hook", h)
    mod.get_axon_ntff_profile_hook = lambda: mod._hook
    sys.modules["antenv.axon_hooks"] = mod
    try:
        from trn_agent_boot.trn_boot import _ntff_profile_via_ctypes
        mod.set_axon_ntff_profile_hook(
            _ntff_profile_via_ctypes("/opt/axon/libaxon_pjrt.so"))
    except Exception:
        pass
    try:
        import concourse.bass_utils as bu
        bu.upload_artifacts = lambda tmpdir: tmpdir
    except Exception:
        pass


_NC_CACHE = {}


def _build_nc(NB):
    key = NB
    if key in _NC_CACHE:
        return _NC_CACHE[key]
    from contextlib import ExitStack
    from concourse import bacc
    nc = bacc.Bacc("TRN2", target_bir_lowering=False, debug=False,
                   num_devices=N_CORES)
    G = 128
    BCk = G * NB
    dt = nc.dram_tensor
    io = {}
    def din(name, shape, dtype):
        io[name] = dt(name, shape, dtype, kind="ExternalInput").ap()
    din("x_t", (IN_DIM, BCk * J), BF16)
    din("rhs_const", (25, J * G), BF16)
    din("W0p", (28, H), BF16)
    din("WS0p", (28, 8), BF16)
    din("GS", (L, L, H, H), BF16)
    din("WSP", (L, L, H, 8), BF16)
    din("XBIAS", (L, H), F32)
    din("W1S", (L, H, H), BF16)
    din("RW1p", (28, H), BF16)
    din("B1", (H,), F32)
    din("W2p", (H // 2, OUT_DIM), BF16)
    din("C2B2", (2, OUT_DIM), F32)
    io["y"] = dt("y", (BCk, J, OUT_DIM), F32, kind="ExternalOutput").ap()
    with tile.TileContext(nc) as tc:
        with ExitStack() as ctx:
            kernel_body(ctx, tc, io, NB)
    nc.compile()
    _NC_CACHE[key] = nc
    return nc


def make_in_maps(inputs, NB=NB_FULL):
    import ml_dtypes
    bf = ml_dtypes.bfloat16
    pp = host_prep(inputs)
    G = 128
    BCk = G * NB
    x = np.asarray(inputs["x"], dtype=np.float32)
    shared = dict(
        rhs_const=np.ascontiguousarray(make_rhs_const(G)),
        W0p=np.ascontiguousarray(pp["W0p"].astype(bf)),
        WS0p=np.ascontiguousarray(pp["WS0p"].astype(bf)),
        GS=np.ascontiguousarray(pp["GS"].astype(bf)),
        WSP=np.ascontiguousarray(pp["WSP"].astype(bf)),
        XBIAS=np.ascontiguousarray(pp["XBIAS"].astype(np.float32)),
        W1S=np.ascontiguousarray(pp["W1S"].astype(bf)),
        RW1p=np.ascontiguousarray(pp["RW1p"].astype(bf)),
        B1=np.ascontiguousarray(pp["b1"].astype(np.float32)),
        W2p=np.ascontiguousarray(pp["W2p"].astype(bf)),
        C2B2=np.ascontiguousarray(
            np.stack([pp["c2"], pp["b2p"]]).astype(np.float32)),
    )
    in_maps = []
    for core in range(N_CORES):
        xc = x[core * BCk:(core + 1) * BCk]
        m = dict(shared)
        m["x_t"] = np.ascontiguousarray(make_x_t(xc, G))
        in_maps.append(m)
    return in_maps


def run_on_cores(inputs, NB=NB_FULL, trace=False):
    _install_ntff_shim()
    from concourse.bass_utils import run_bass_kernel_spmd
    nc = _build_nc(NB)
    in_maps = make_in_maps(inputs, NB)
    res = run_bass_kernel_spmd(nc, in_maps, list(range(N_CORES)), trace=trace)
    ys = [res.results[c]["y"] for c in range(N_CORES)]
    y = np.concatenate(ys, axis=0).astype(np.float32)
    return y, res


def kernel(**inputs) -> np.ndarray:
    y, _ = run_on_cores(inputs)
    return y
